# revision 17
# baseline (speedup 1.0000x reference)
"""Relative-position attention (Music-Transformer style skew) + LayerNorm,
distributed over 8 TRN2 NeuronCores.

Sharding: data-parallel over batch (B=4) x tensor-parallel over head-halves
(H=8 -> 2 groups of 4). Core c handles batch b=c//2, heads [4*(c%2), 4*(c%2)+4),
producing output channels [256*(c%2), +256) of y[b]. The final LayerNorm needs
full-E stats, exchanged via a tiny pairwise AllReduce of (sum, sumsq) per
512-row chunk, overlapped with the next chunk's compute.

Skew trick: Srel[i,j] = F[(i+1)*S + j] where F is the row-major flat view of
the padded matrix P[i, 0]=0, P[i, 1+l]=QEr[i, l] (P is [S, S+1]). We bounce P
through DRAM in fp8-e4m3; the skewed read back is a plain strided DMA.

PE strategy: everything on the PE is bf16/fp8 (enables FWL weight loads and
1 cycle/row at every clock state; fp32 runs multi-pass).  QEr and QK have
contraction dim 64 (head dim), so the two heads of a 128-partition tile are
issued back-to-back to different row groups (base partitions 0/64) and stream
through the array concurrently (~2x).  Srel is injected into the transposed
score PSUM via fp8 identity matmuls (fused transpose + add).  AV uses flipped
operand roles: the V block (65 cols incl. a ones column for the softmax
denominator) is PE-stationary and the exp'd transposed scores stream at N=512.
"""

import numpy as np

import concourse.bass as bass
import concourse.mybir as mybir
from concourse import masks
from concourse.tile import TileContext

F32 = mybir.dt.float32
BF16 = mybir.dt.bfloat16
FP8 = mybir.dt.float8e4

B, S, E, H = 4, 2048, 512, 8
HD = E // H          # 64
HLOC = 4             # heads per core
CH = HLOC * HD       # 256 output channels per core
SCALE = float(E) ** -0.5
EPS = 1e-5
N_CORES = 8
QG = 4               # 128-row q-blocks per 512-query group


def build_nc(s=S, n_cores=N_CORES, debug=False, legalize=True):
    """Build the per-core Bass graph (SPMD: same graph on all cores)."""
    nc = bass.Bass(target_bir_lowering=False, debug=debug)

    SB = s // 128        # number of 128-row blocks
    KC = s // 512        # number of 512-col chunks
    NSSB = SB // QG      # number of 512-query groups
    LT = min(1024, s)    # phase-A psum tile width (2 banks)
    NLT = s // LT

    x_d = nc.declare_dram_parameter("x", [s, E], F32, isOutput=False)
    wq_d = nc.declare_dram_parameter("wq", [CH, E], F32, isOutput=False)
    wk_d = nc.declare_dram_parameter("wk", [CH, E], F32, isOutput=False)
    wv_d = nc.declare_dram_parameter("wv", [CH, E], F32, isOutput=False)
    er_d = nc.declare_dram_parameter("er", [s, HD], F32, isOutput=False)
    gamma_d = nc.declare_dram_parameter("gamma", [1, CH], F32, isOutput=False)
    beta_d = nc.declare_dram_parameter("beta", [1, CH], F32, isOutput=False)
    out_d = nc.declare_dram_parameter("out", [s, CH], F32, isOutput=True)

    # Padded-QEr bounce buffers, one per head, flat [S*(S+1)] fp8.
    p_ds = [nc.dram_tensor(f"pbuf{h}", [s * (s + 1)], FP8)
            for h in range(HLOC)]
    cc_in = nc.dram_tensor("cc_in", [s, 2], F32)
    cc_out = nc.dram_tensor("cc_out", [s, 2], F32)

    pairs = [[2 * i, 2 * i + 1] for i in range(n_cores // 2)]

    with TileContext(nc) as tc:
        with (
            tc.tile_pool(name="const", bufs=1) as const_pool,
            tc.tile_pool(name="persist", bufs=1) as pp,
        ):
            ident_bf16 = const_pool.tile([128, 128], BF16)
            ident_fp8 = const_pool.tile([128, 128], FP8)
            masks.make_identity(nc, ident_bf16[:])
            masks.make_identity(nc, ident_fp8[:])
            gamma_bc = const_pool.tile([128, CH], F32)
            beta_bc = const_pool.tile([128, CH], F32)

            # ---- persistent SBUF tensors (all bf16 on the PE paths) ----
            # ErT replicated into both partition halves so each head of a
            # pair finds it at its own base partition.
            erT = pp.tile([128, s], BF16, tag="erT")
            qT = [pp.tile([128, s], BF16, tag=f"qT{oc}", name=f"qT{oc}")
                  for oc in range(2)]
            kT = [pp.tile([128, s], BF16, tag=f"kT{oc}", name=f"kT{oc}")
                  for oc in range(2)]
            # v with a ones column appended per head: [128, HLOC*(HD+1)] bf16
            vaug = [pp.tile([128, HLOC * (HD + 1)], BF16, tag=f"va{sb}",
                            name=f"va{sb}") for sb in range(SB)]
            # raw AV numerators + softmax sum per head (65-col head slots,
            # matching the avT transpose output so it lands in one copy)
            outp = [pp.tile([128, HLOC * (HD + 1)], F32, tag=f"op{sb}",
                            name=f"op{sb}") for sb in range(SB)]

            # ====== setup + projections (xT/wT freed afterwards) ======
            xw_pool = tc.tile_pool(name="xw", bufs=1)
            xwp = xw_pool.__enter__()
            xT = [xwp.tile([128, s], BF16, tag=f"xT{ec}", name=f"xT{ec}")
                  for ec in range(4)]
            wT = {
                w: [xwp.tile([128, CH], BF16, tag=f"{w}T{ec}",
                             name=f"{w}T{ec}") for ec in range(4)]
                for w in ("wq", "wk", "wv")
            }
            with (
                tc.tile_pool(name="ld", bufs=4) as ld_pool,
                tc.tile_pool(name="cst", bufs=4) as cst_pool,
                tc.tile_pool(name="ps_set", bufs=4, space="PSUM") as ps_set,
            ):
                # Warm-up: absorb the Pool (identity-creation) dependency
                # into PE's observed clock.
                warm = ps_set.tile([128, 128], F32, tag="pset")
                nc.tensor.matmul(
                    warm[:], ident_bf16[:], ident_bf16[:],
                    start=True, stop=True)

                # xT[ec][:, i*128:(i+1)*128] = bf16(x[i-block, ec-block]).T
                for sb in range(SB):
                    xt = ld_pool.tile([128, E], F32, tag="xld")
                    nc.sync.dma_start(xt[:], x_d[sb * 128:(sb + 1) * 128, :])
                    xb = cst_pool.tile([128, E], BF16, tag="xcst")
                    nc.vector.tensor_copy(xb[:], xt[:])
                    for ec in range(4):
                        pst = ps_set.tile([128, 128], F32, tag="pset")
                        nc.tensor.matmul(
                            pst[:], xb[:, ec * 128:(ec + 1) * 128],
                            ident_bf16[:], start=True, stop=True)
                        eng = nc.scalar.copy if ec % 2 else \
                            nc.vector.tensor_copy
                        eng(xT[ec][:, sb * 128:(sb + 1) * 128], pst[:])
                # weights
                for w_name, w_d in (("wq", wq_d), ("wk", wk_d), ("wv", wv_d)):
                    for pc in range(CH // 128):
                        wt = ld_pool.tile([128, E], F32, tag="wld")
                        nc.sync.dma_start(
                            wt[:], w_d[pc * 128:(pc + 1) * 128, :])
                        wb = cst_pool.tile([128, E], BF16, tag="wcst")
                        nc.vector.tensor_copy(wb[:], wt[:])
                        for ec in range(4):
                            pst = ps_set.tile([128, 128], F32, tag="pset")
                            nc.tensor.matmul(
                                pst[:], wb[:, ec * 128:(ec + 1) * 128],
                                ident_bf16[:], start=True, stop=True)
                            eng = nc.scalar.copy if ec % 2 else \
                                nc.vector.tensor_copy
                            eng(wT[w_name][ec][:, pc * 128:(pc + 1) * 128],
                                pst[:])
                # Er: one load+cast, then transpose into both partition halves
                et = ld_pool.tile([128, SB * HD], F32, tag="eld")
                nc.sync.dma_start(
                    et[:].rearrange("p (b d) -> p b d", d=HD),
                    er_d[:].rearrange("(b p) d -> p b d", p=128))
                eb = cst_pool.tile([128, SB * HD], BF16, tag="ecst")
                nc.vector.tensor_copy(eb[:], et[:])
                for sb in range(SB):
                    pst = ps_set.tile([128, 128], F32, tag="psete")
                    nc.tensor.matmul(
                        pst[0:64, :], eb[:, sb * HD:(sb + 1) * HD],
                        ident_bf16[:], start=True, stop=True)
                    nc.tensor.matmul(
                        pst[64:128, :], eb[:, sb * HD:(sb + 1) * HD],
                        ident_bf16[:], start=True, stop=True)
                    nc.vector.tensor_copy(
                        erT[:, sb * 128:(sb + 1) * 128], pst[:])

            # ================= projections =================
            with tc.tile_pool(name="ps_pj", bufs=4, space="PSUM") as ps_pj:
                # qT / kT: [oc*128+p, t] = sum_e W[oc*128+p, e] x[t, e]
                for dst, w_name in ((qT, "wq"), (kT, "wk")):
                    for oc in range(2):
                        for sc in range(KC):
                            ps = ps_pj.tile([128, 512], F32, tag="pj")
                            for ec in range(4):
                                nc.tensor.matmul(
                                    ps[:],
                                    wT[w_name][ec][:, oc * 128:
                                                   (oc + 1) * 128],
                                    xT[ec][:, sc * 512:(sc + 1) * 512],
                                    start=(ec == 0), stop=(ec == 3))
                            eng = nc.scalar.copy if sc % 2 else \
                                nc.vector.tensor_copy
                            eng(dst[oc][:, sc * 512:(sc + 1) * 512], ps[:])
                # v natural + ones column, bf16
                for sb in range(SB):
                    ps = ps_pj.tile([128, CH], F32, tag="pj")
                    for ec in range(4):
                        nc.tensor.matmul(
                            ps[:],
                            xT[ec][:, sb * 128:(sb + 1) * 128],
                            wT["wv"][ec][:],
                            start=(ec == 0), stop=(ec == 3))
                    for h in range(HLOC):
                        nc.scalar.copy(
                            vaug[sb][:, h * (HD + 1):h * (HD + 1) + HD],
                            ps[:, h * HD:(h + 1) * HD])
                        nc.vector.memset(
                            vaug[sb][:, h * (HD + 1) + HD:
                                     (h + 1) * (HD + 1)],
                            1.0)
            xw_pool.__exit__(None, None, None)
            # LN constants, needed only at the tail of each chunk
            nc.sync.dma_start(gamma_bc[:], gamma_d[:].broadcast_to((128, CH)))
            nc.sync.dma_start(beta_bc[:], beta_d[:].broadcast_to((128, CH)))

            # ================= attention =================
            with (
                tc.tile_pool(name="wrk", bufs=4) as wrk,
                tc.tile_pool(name="wrk2", bufs=4) as wrk2,
                tc.tile_pool(name="ttp", bufs=4) as ttp,
                tc.tile_pool(name="avp", bufs=2) as avp,
                tc.tile_pool(name="small", bufs=8) as small,
                tc.tile_pool(name="ps_m", bufs=3, space="PSUM") as ps_m,
                tc.tile_pool(name="ps_av", bufs=1, space="PSUM") as ps_av,
                tc.tile_pool(name="ps_avT", bufs=1, space="PSUM") as ps_avT,
            ):
                def phase_a_pair(hp, sb):
                    """QEr for heads (2hp, 2hp+1), q-block sb -> P[h] (fp8).

                    The two heads' matmuls go to different row groups
                    (base partitions 0/64) and stream concurrently.
                    Column 0 of every P row was pre-zeroed at startup, so
                    only the s QEr columns are written here."""
                    pex = [wrk.tile([128, s + 1], FP8, tag=f"pexp{i}",
                                    name=f"pexp{i}") for i in range(2)]
                    # P's zero column travels with the tile: the DRAM write
                    # is then fully contiguous (no per-row scatter)
                    nc.vector.memset(pex[0][:, 0:1], 0.0)
                    nc.vector.memset(pex[1][:, 0:1], 0.0)
                    for lt in range(NLT):
                        psA = ps_m.tile([128, LT], F32, tag="pm", name="psA")
                        psB = ps_m.tile([128, LT], F32, tag="pm", name="psB")
                        for c in range(LT // 512):
                            l0 = lt * LT + c * 512
                            nc.tensor.matmul(
                                psA[:, c * 512:(c + 1) * 512],
                                qT[hp][0:64, sb * 128:(sb + 1) * 128],
                                erT[0:64, l0:l0 + 512],
                                start=True, stop=True)
                            nc.tensor.matmul(
                                psB[:, c * 512:(c + 1) * 512],
                                qT[hp][64:128, sb * 128:(sb + 1) * 128],
                                erT[64:128, l0:l0 + 512],
                                start=True, stop=True)
                        # drains: DVE for head A; head B alternates DVE/ACT
                        # (GPSIMD cannot read PSUM on TRN2)
                        nc.vector.tensor_copy(
                            pex[0][:, 1 + lt * LT:1 + (lt + 1) * LT], psA[:])
                        engb = nc.vector.tensor_copy if sb % 2 == 0 else \
                            nc.scalar.copy
                        engb(pex[1][:, 1 + lt * LT:1 + (lt + 1) * LT],
                             psB[:])
                    for i in range(2):
                        h = 2 * hp + i
                        base1 = sb * 128 * (s + 1)
                        nc.sync.dma_start(
                            p_ds[h][base1:base1 + 128 * (s + 1)]
                            .rearrange("(r c) -> r c", c=s + 1),
                            pex[i][:])

                def skew_read(h, sb):
                    """Skewed strided read of P[h] for q-block sb."""
                    st = wrk2.tile([128, s], FP8, tag="srel", name="srel",
                                   bufs=22)
                    base = (sb * 128 + 1) * s
                    nc.sync.dma_start(
                        st[:],
                        p_ds[h][base:base + 128 * s]
                        .rearrange("(r c) -> r c", c=s))
                    return st

                def tp_step(hp, ssb, tp, srels, ttA, ttB):
                    """Transposed scores for one pair of t-blocks, both
                    heads of pair hp.

                    scoresT[t-block, i] = k_tb^T q (kT-block stationary, q
                    moving at N=512); the heads alternate row groups so the
                    two QK matmuls overlap in the array.  The four srel
                    skew-blocks are transpose-accumulated into the same PSUM
                    group via fp8 identity matmuls; exp drains PSUM straight
                    into the AV-ready ttile slices, [128,1024] per ACT
                    instruction."""
                    q0 = ssb * QG * 128
                    psA = ps_m.tile([128, 1024], F32, tag="pm", name="psA")
                    psB = ps_m.tile([128, 1024], F32, tag="pm", name="psB")
                    for j in range(2):
                        tb = 2 * tp + j
                        nc.tensor.matmul(
                            psA[:, j * 512:(j + 1) * 512],
                            kT[hp][0:64, tb * 128:(tb + 1) * 128],
                            qT[hp][0:64, q0:q0 + 512],
                            start=True, stop=False, skip_group_check=True)
                        nc.tensor.matmul(
                            psB[:, j * 512:(j + 1) * 512],
                            kT[hp][64:128, tb * 128:(tb + 1) * 128],
                            qT[hp][64:128, q0:q0 + 512],
                            start=True, stop=False, skip_group_check=True)
                    for j in range(2):
                        tb = 2 * tp + j
                        for ps, sr in ((psA, srels[0]), (psB, srels[1])):
                            for ib in range(QG):
                                nc.tensor.matmul(
                                    ps[:, j * 512 + ib * 128:
                                       j * 512 + (ib + 1) * 128],
                                    sr[ib][:, tb * 128:(tb + 1) * 128],
                                    ident_fp8[:],
                                    start=False, stop=(ib == QG - 1),
                                    skip_group_check=True)
                    nc.scalar.activation(
                        ttA[:, 2 * tp:2 * tp + 2, :]
                        .rearrange("p a c -> p (a c)"),
                        psA[:],
                        mybir.ActivationFunctionType.Exp, scale=SCALE)
                    nc.scalar.activation(
                        ttB[:, 2 * tp:2 * tp + 2, :]
                        .rearrange("p a c -> p (a c)"),
                        psB[:],
                        mybir.ActivationFunctionType.Exp, scale=SCALE)

                def phase_b_av(hloc, ssb, ttile):
                    """AV burst for one 512-query group; stash raw result."""
                    av_ps = ps_av.tile([HD + 1, 512], F32, tag="av",
                                       name="av_ps")
                    for ci in range(SB):
                        nc.tensor.matmul(
                            av_ps[:],
                            vaug[ci][:, hloc * (HD + 1):
                                     (hloc + 1) * (HD + 1)],
                            ttile[:, ci, :],
                            start=(ci == 0), stop=(ci == SB - 1))
                    avs = avp.tile([HD + 1, 512], BF16, tag="avs",
                                   name="avs")
                    nc.scalar.copy(avs[:], av_ps[:])
                    for q4 in range(QG):
                        sb = ssb * QG + q4
                        avT = ps_avT.tile([128, HD + 1], F32, tag="avT",
                                          name="avT")
                        nc.tensor.matmul(
                            avT[:], avs[:, q4 * 128:(q4 + 1) * 128],
                            ident_bf16[0:HD + 1, 0:HD + 1],
                            start=True, stop=True)
                        nc.vector.tensor_copy(
                            outp[sb][:, hloc * (HD + 1):
                                     (hloc + 1) * (HD + 1)],
                            avT[:])

                def div_block(sb):
                    """One batched reciprocal of the 4 heads' softmax sums
                    (at the 65-col slot tails), then scale the raw AV
                    numerators in place."""
                    o3 = outp[sb][:].rearrange("p (h c) -> p h c", c=HD + 1)
                    rinv4 = small.tile([128, HLOC], F32, tag="rinv4",
                                       name="rinv4")
                    nc.vector.reciprocal(rinv4[:], o3[:, :, HD])
                    for hh in range(HLOC):
                        nc.vector.tensor_scalar_mul(
                            o3[:, hh, 0:HD],
                            o3[:, hh, 0:HD],
                            rinv4[:, hh:hh + 1])

                def ln_stats_block(sb):
                    o3 = outp[sb][:].rearrange("p (h c) -> p h c", c=HD + 1)
                    s1 = small.tile([128, 1], F32, tag="s1", name="s1")
                    nc.vector.reduce_sum(
                        s1[:], o3[:, :, 0:HD], axis=mybir.AxisListType.XY)
                    sq = small.tile([128, 1], F32, tag="sq", name="sq")
                    scr = wrk.tile([128, CH], F32, tag="scr", name="scr")
                    nc.scalar.activation(
                        scr[:], o3[:, :, 0:HD],
                        mybir.ActivationFunctionType.Square, accum_out=sq[:])
                    nc.sync.dma_start(
                        cc_in[sb * 128:(sb + 1) * 128, 0:1], s1[:])
                    nc.sync.dma_start(
                        cc_in[sb * 128:(sb + 1) * 128, 1:2], sq[:])

                def ln_chunk_collective(ssb):
                    r0 = ssb * QG * 128
                    r1 = (ssb + 1) * QG * 128
                    nc.gpsimd.collective_compute(
                        "AllReduce", mybir.AluOpType.add,
                        replica_groups=pairs,
                        ins=[cc_in[r0:r1, :].opt()],
                        outs=[cc_out[r0:r1, :].opt()])

                def ln_apply_block(sb):
                    o3 = outp[sb][:].rearrange("p (h c) -> p h c", c=HD + 1)
                    st = small.tile([128, 2], F32, tag="st")
                    nc.sync.dma_start(
                        st[:], cc_out[sb * 128:(sb + 1) * 128, :])
                    me2 = small.tile([128, 2], F32, tag="me2")
                    nc.vector.tensor_scalar_mul(me2[:], st[:], 1.0 / E)
                    msq = small.tile([128, 1], F32, tag="msq")
                    nc.vector.tensor_mul(msq[:], me2[:, 0:1], me2[:, 0:1])
                    var = small.tile([128, 1], F32, tag="var")
                    nc.vector.tensor_scalar(
                        var[:], me2[:, 1:2], msq[:], EPS,
                        op0=mybir.AluOpType.subtract,
                        op1=mybir.AluOpType.add)
                    vrec = small.tile([128, 1], F32, tag="vrec")
                    nc.vector.reciprocal(vrec[:], var[:])
                    rstd = small.tile([128, 1], F32, tag="rstd")
                    nc.scalar.activation(
                        rstd[:], vrec[:],
                        mybir.ActivationFunctionType.Sqrt)
                    tmp = wrk.tile([128, CH], F32, tag="tmp")
                    nc.vector.tensor_scalar(
                        tmp[:].rearrange("p (h c) -> p h c", c=HD),
                        o3[:, :, 0:HD], me2[:, 0:1], rstd[:],
                        op0=mybir.AluOpType.subtract,
                        op1=mybir.AluOpType.mult)
                    y1 = wrk2.tile([128, CH], F32, tag="y1")
                    nc.gpsimd.tensor_mul(y1[:], tmp[:], gamma_bc[:])
                    y2 = wrk.tile([128, CH], F32, tag="y2")
                    nc.gpsimd.tensor_add(y2[:], y1[:], beta_bc[:])
                    nc.sync.dma_start(
                        out_d[sb * 128:(sb + 1) * 128, :], y2[:])

                # -------- main attention loop: ssb outer, heads inner ------
                # Software pipeline: the next group's QEr (phase A, PE-light
                # + DVE/ACT drains) is interleaved between this group's
                # score tp-steps (PE-dense, ACT exp drains) so every engine
                # sees work from two streams at all times.  AV bursts for
                # pair 0 run inside pair 1's score loop.
                NTP = SB // 2
                for hp in range(2):
                    for sb4 in range(QG):
                        phase_a_pair(hp, sb4)
                # early skew reads for heads 0/1 of the first group
                nxt_early = {h: [skew_read(h, sb4) for sb4 in range(QG - 1)]
                             for h in range(2)}
                for ssb in range(NSSB):
                    nxt = ssb + 1 < NSSB
                    nb = (ssb + 1) * QG
                    srels = nxt_early
                    # heads 2/3's early reads: their tp-steps run late
                    # enough that issuing here still hides the DMA
                    for h in range(2, HLOC):
                        srels[h] = [skew_read(h, ssb * QG + sb4)
                                    for sb4 in range(QG - 1)]
                    tts = {}
                    for hp in range(2):
                        if nxt:
                            phase_a_pair(hp, nb)
                        # last skew read: needs P row nb*128 (written above)
                        for i in range(2):
                            srels[2 * hp + i].append(
                                skew_read(2 * hp + i, ssb * QG + QG - 1))
                        ttA = ttp.tile([128, SB, 512], BF16, tag="tt",
                                       name="ttA")
                        ttB = ttp.tile([128, SB, 512], BF16, tag="tt",
                                       name="ttB")
                        tts[hp] = (ttA, ttB)
                        pend_a = list(range(nb + 1, nb + QG)) if nxt else []
                        av_done = [False, False]
                        for tp in range(NTP):
                            tp_step(hp, ssb,  tp,
                                    (srels[2 * hp], srels[2 * hp + 1]),
                                    ttA, ttB)
                            if tp % 2 == 1 and pend_a:
                                phase_a_pair(hp, pend_a.pop(0))
                            if hp == 1 and tp == 2:
                                phase_b_av(0, ssb, tts[0][0])
                                av_done[0] = True
                            if hp == 1 and tp == 5:
                                phase_b_av(1, ssb, tts[0][1])
                                av_done[1] = True
                        while pend_a:
                            phase_a_pair(hp, pend_a.pop(0))
                        if hp == 0:
                            # previous chunk's LN here: by now its
                            # AllReduce has had half an iteration to land,
                            # so the DVE queue isn't blocked at its head
                            if ssb > 0:
                                for sb4 in range(QG):
                                    ln_apply_block((ssb - 1) * QG + sb4)
                        else:
                            if not av_done[0]:
                                phase_b_av(0, ssb, tts[0][0])
                            if not av_done[1]:
                                phase_b_av(1, ssb, tts[0][1])
                    phase_b_av(2, ssb, tts[1][0])
                    phase_b_av(3, ssb, tts[1][1])
                    for sb4 in range(QG):
                        div_block(ssb * QG + sb4)
                        ln_stats_block(ssb * QG + sb4)
                    ln_chunk_collective(ssb)
                    # prefetch heads 0/1's skew reads for the next group
                    # (their P rows were written during this iteration)
                    nxt_early = {}
                    if nxt:
                        nxt_early = {
                            h: [skew_read(h, nb + sb4)
                                for sb4 in range(QG - 1)]
                            for h in range(2)}
                for sb4 in range(QG):
                    ln_apply_block((NSSB - 1) * QG + sb4)

    if legalize:
        _legalize_waits(nc)
    return nc


def _legalize_waits(nc):
    """walrus's codegen accepts at most one sync wait on most instruction
    structs; hoist extra waits onto NoOps inserted just before, on the
    same engine queue (program order preserves the semantics)."""
    n = 0
    keep = set()
    for bb in nc.main_func.blocks:
        out = []
        for inst in bb.instructions:
            si = inst.sync_info
            if (inst.opcode not in keep and si is not None
                    and si.on_wait and len(si.on_wait) > 1):
                for w in si.on_wait[:-1]:
                    nop = mybir.InstNoOp(
                        name=f"I-mmw{n}", ins=[], outs=[])
                    n += 1
                    nop.engine = inst.engine
                    nop.sync_info = mybir.SyncInfo(
                        on_wait=[w], on_update=[])
                    out.append(nop)
                si.on_wait = [si.on_wait[-1]]
            out.append(inst)
        bb.instructions = out


_NC_CACHE = {}


def _get_nc(s=S, n_cores=N_CORES):
    key = (s, n_cores)
    if key not in _NC_CACHE:
        _NC_CACHE[key] = build_nc(s, n_cores)
    return _NC_CACHE[key]


def make_in_maps(x, Wq, Wk, Wv, Er, gamma, beta, n_cores=N_CORES):
    in_maps = []
    for c in range(n_cores):
        b, hg = c // 2, c % 2
        sl = slice(hg * CH, (hg + 1) * CH)
        in_maps.append({
            "x": np.ascontiguousarray(x[b], dtype=np.float32),
            "wq": np.ascontiguousarray(Wq[sl], dtype=np.float32),
            "wk": np.ascontiguousarray(Wk[sl], dtype=np.float32),
            "wv": np.ascontiguousarray(Wv[sl], dtype=np.float32),
            "er": np.ascontiguousarray(Er, dtype=np.float32),
            "gamma": np.ascontiguousarray(gamma[sl], dtype=np.float32)[None, :],
            "beta": np.ascontiguousarray(beta[sl], dtype=np.float32)[None, :],
        })
    return in_maps


def assemble(results, n_cores=N_CORES, s=S):
    y = np.empty((n_cores // 2, s, E), np.float32)
    for c in range(n_cores):
        y[c // 2, :, (c % 2) * CH:(c % 2 + 1) * CH] = results[c]["out"]
    return y


def kernel(**inputs):
    from concourse.bass_utils import run_bass_kernel_spmd
    nc = _get_nc()
    in_maps = make_in_maps(
        inputs["x"], inputs["Wq"], inputs["Wk"], inputs["Wv"],
        inputs["Er"], inputs["gamma"], inputs["beta"])
    res = run_bass_kernel_spmd(nc, in_maps, list(range(N_CORES)))
    return assemble(res.results)


# revision 22
# speedup vs baseline: 1.1451x; 1.1451x over previous
"""Relative-position attention (Music-Transformer style skew) + LayerNorm,
distributed over 8 TRN2 NeuronCores.

Sharding: data-parallel over batch (B=4) x tensor-parallel over head-halves
(H=8 -> 2 groups of 4). Core c handles batch b=c//2, heads [4*(c%2), 4*(c%2)+4),
producing output channels [256*(c%2), +256) of y[b]. The final LayerNorm needs
full-E stats, exchanged via a tiny pairwise AllReduce of (sum, sumsq) per
512-row chunk, overlapped with the next chunk's compute.

Skew trick: Srel[i,j] = F[(i+1)*S + j] where F is the row-major flat view of
the padded matrix P[i, 0]=0, P[i, 1+l]=QEr[i, l] (P is [S, S+1]). We bounce P
through DRAM in fp8-e4m3; the skewed read back is a plain strided DMA.

PE strategy: everything on the PE is bf16/fp8 (enables FWL weight loads and
1 cycle/row at every clock state; fp32 runs multi-pass).  QEr and QK have
contraction dim 64 (head dim), so the two heads of a 128-partition tile are
issued back-to-back to different row groups (base partitions 0/64) and stream
through the array concurrently (~2x).  Srel is injected into the transposed
score PSUM via fp8 identity matmuls (fused transpose + add).  AV uses flipped
operand roles: the V block (65 cols incl. a ones column for the softmax
denominator) is PE-stationary and the exp'd transposed scores stream at N=512.
"""

import numpy as np

import concourse.bass as bass
import concourse.mybir as mybir
from concourse import masks
from concourse.tile import TileContext

F32 = mybir.dt.float32
BF16 = mybir.dt.bfloat16
FP8 = mybir.dt.float8e4

B, S, E, H = 4, 2048, 512, 8
HD = E // H          # 64
HLOC = 4             # heads per core
CH = HLOC * HD       # 256 output channels per core
SCALE = float(E) ** -0.5
EPS = 1e-5
N_CORES = 8
QG = 4               # 128-row q-blocks per 512-query group


def build_nc(s=S, n_cores=N_CORES, debug=False, legalize=True):
    """Build the per-core Bass graph (SPMD: same graph on all cores)."""
    nc = bass.Bass(target_bir_lowering=False, debug=debug)

    SB = s // 128        # number of 128-row blocks
    KC = s // 512        # number of 512-col chunks
    NSSB = SB // QG      # number of 512-query groups
    LT = min(1024, s)    # phase-A psum tile width (2 banks)
    NLT = s // LT

    x_d = nc.declare_dram_parameter("x", [s, E], F32, isOutput=False)
    wq_d = nc.declare_dram_parameter("wq", [CH, E], F32, isOutput=False)
    wk_d = nc.declare_dram_parameter("wk", [CH, E], F32, isOutput=False)
    wv_d = nc.declare_dram_parameter("wv", [CH, E], F32, isOutput=False)
    er_d = nc.declare_dram_parameter("er", [s, HD], F32, isOutput=False)
    gamma_d = nc.declare_dram_parameter("gamma", [1, CH], F32, isOutput=False)
    beta_d = nc.declare_dram_parameter("beta", [1, CH], F32, isOutput=False)
    out_d = nc.declare_dram_parameter("out", [s, CH], F32, isOutput=True)

    # Padded-QEr bounce buffers, one per head, flat [S*(S+1)] fp8.
    p_ds = [nc.dram_tensor(f"pbuf{h}", [s * (s + 1)], FP8)
            for h in range(HLOC)]
    cc_in = nc.dram_tensor("cc_in", [s, 2], F32)
    cc_out = nc.dram_tensor("cc_out", [s, 2], F32)

    pairs = [[2 * i, 2 * i + 1] for i in range(n_cores // 2)]

    with TileContext(nc) as tc:
        with (
            tc.tile_pool(name="const", bufs=1) as const_pool,
            tc.tile_pool(name="persist", bufs=1) as pp,
        ):
            ident_bf16 = const_pool.tile([128, 128], BF16)
            ident_fp8 = const_pool.tile([128, 128], FP8)
            masks.make_identity(nc, ident_bf16[:])
            masks.make_identity(nc, ident_fp8[:])
            gamma_bc = const_pool.tile([128, CH], F32)
            beta_bc = const_pool.tile([128, CH], F32)

            # ---- persistent SBUF tensors (all bf16 on the PE paths) ----
            # ErT replicated into both partition halves so each head of a
            # pair finds it at its own base partition.
            erT = pp.tile([128, s], BF16, tag="erT")
            qT = [pp.tile([128, s], BF16, tag=f"qT{oc}", name=f"qT{oc}")
                  for oc in range(2)]
            kT = [pp.tile([128, s], BF16, tag=f"kT{oc}", name=f"kT{oc}")
                  for oc in range(2)]
            # v with a ones column appended per head: [128, HLOC*(HD+1)] bf16
            vaug = [pp.tile([128, HLOC * (HD + 1)], BF16, tag=f"va{sb}",
                            name=f"va{sb}") for sb in range(SB)]
            # raw AV numerators + softmax sum per head (65-col head slots,
            # matching the avT transpose output so it lands in one copy)
            outp = [pp.tile([128, HLOC * (HD + 1)], F32, tag=f"op{sb}",
                            name=f"op{sb}") for sb in range(SB)]

            # ====== setup + projections (xT/wT freed afterwards) ======
            xw_pool = tc.tile_pool(name="xw", bufs=1)
            xwp = xw_pool.__enter__()
            xT = [xwp.tile([128, s], BF16, tag=f"xT{ec}", name=f"xT{ec}")
                  for ec in range(4)]
            wT = {
                w: [xwp.tile([128, CH], BF16, tag=f"{w}T{ec}",
                             name=f"{w}T{ec}") for ec in range(4)]
                for w in ("wq", "wk", "wv")
            }
            with (
                tc.tile_pool(name="ld", bufs=4) as ld_pool,
                tc.tile_pool(name="cst", bufs=4) as cst_pool,
                tc.tile_pool(name="ps_set", bufs=4, space="PSUM") as ps_set,
            ):
                # Warm-up: absorb the Pool (identity-creation) dependency
                # into PE's observed clock.
                warm = ps_set.tile([128, 128], F32, tag="pset")
                nc.tensor.matmul(
                    warm[:], ident_bf16[:], ident_bf16[:],
                    start=True, stop=True)

                # xT[ec][:, i*128:(i+1)*128] = bf16(x[i-block, ec-block]).T
                for sb in range(SB):
                    xt = ld_pool.tile([128, E], F32, tag="xld")
                    nc.sync.dma_start(xt[:], x_d[sb * 128:(sb + 1) * 128, :])
                    xb = cst_pool.tile([128, E], BF16, tag="xcst")
                    nc.vector.tensor_copy(xb[:], xt[:])
                    for ec in range(4):
                        pst = ps_set.tile([128, 128], F32, tag="pset")
                        nc.tensor.matmul(
                            pst[:], xb[:, ec * 128:(ec + 1) * 128],
                            ident_bf16[:], start=True, stop=True)
                        eng = nc.scalar.copy if ec % 2 else \
                            nc.vector.tensor_copy
                        eng(xT[ec][:, sb * 128:(sb + 1) * 128], pst[:])
                # weights
                for w_name, w_d in (("wq", wq_d), ("wk", wk_d), ("wv", wv_d)):
                    for pc in range(CH // 128):
                        wt = ld_pool.tile([128, E], F32, tag="wld")
                        nc.sync.dma_start(
                            wt[:], w_d[pc * 128:(pc + 1) * 128, :])
                        wb = cst_pool.tile([128, E], BF16, tag="wcst")
                        nc.vector.tensor_copy(wb[:], wt[:])
                        for ec in range(4):
                            pst = ps_set.tile([128, 128], F32, tag="pset")
                            nc.tensor.matmul(
                                pst[:], wb[:, ec * 128:(ec + 1) * 128],
                                ident_bf16[:], start=True, stop=True)
                            eng = nc.scalar.copy if ec % 2 else \
                                nc.vector.tensor_copy
                            eng(wT[w_name][ec][:, pc * 128:(pc + 1) * 128],
                                pst[:])
                # Er: one load+cast, then transpose into both partition halves
                et = ld_pool.tile([128, SB * HD], F32, tag="eld")
                nc.sync.dma_start(
                    et[:].rearrange("p (b d) -> p b d", d=HD),
                    er_d[:].rearrange("(b p) d -> p b d", p=128))
                eb = cst_pool.tile([128, SB * HD], BF16, tag="ecst")
                nc.vector.tensor_copy(eb[:], et[:])
                for sb in range(SB):
                    pst = ps_set.tile([128, 128], F32, tag="psete")
                    nc.tensor.matmul(
                        pst[0:64, :], eb[:, sb * HD:(sb + 1) * HD],
                        ident_bf16[:], start=True, stop=True)
                    nc.tensor.matmul(
                        pst[64:128, :], eb[:, sb * HD:(sb + 1) * HD],
                        ident_bf16[:], start=True, stop=True)
                    nc.vector.tensor_copy(
                        erT[:, sb * 128:(sb + 1) * 128], pst[:])

            # ================= projections =================
            with tc.tile_pool(name="ps_pj", bufs=4, space="PSUM") as ps_pj:
                # qT / kT: [oc*128+p, t] = sum_e W[oc*128+p, e] x[t, e]
                for dst, w_name in ((qT, "wq"), (kT, "wk")):
                    for oc in range(2):
                        for sc in range(KC):
                            ps = ps_pj.tile([128, 512], F32, tag="pj")
                            for ec in range(4):
                                nc.tensor.matmul(
                                    ps[:],
                                    wT[w_name][ec][:, oc * 128:
                                                   (oc + 1) * 128],
                                    xT[ec][:, sc * 512:(sc + 1) * 512],
                                    start=(ec == 0), stop=(ec == 3))
                            eng = nc.scalar.copy if sc % 2 else \
                                nc.vector.tensor_copy
                            eng(dst[oc][:, sc * 512:(sc + 1) * 512], ps[:])
                # v natural + ones column, bf16
                for sb in range(SB):
                    ps = ps_pj.tile([128, CH], F32, tag="pj")
                    for ec in range(4):
                        nc.tensor.matmul(
                            ps[:],
                            xT[ec][:, sb * 128:(sb + 1) * 128],
                            wT["wv"][ec][:],
                            start=(ec == 0), stop=(ec == 3))
                    for h in range(HLOC):
                        nc.scalar.copy(
                            vaug[sb][:, h * (HD + 1):h * (HD + 1) + HD],
                            ps[:, h * HD:(h + 1) * HD])
                        nc.vector.memset(
                            vaug[sb][:, h * (HD + 1) + HD:
                                     (h + 1) * (HD + 1)],
                            1.0)
            xw_pool.__exit__(None, None, None)
            # LN constants, needed only at the tail of each chunk
            nc.sync.dma_start(gamma_bc[:], gamma_d[:].broadcast_to((128, CH)))
            nc.sync.dma_start(beta_bc[:], beta_d[:].broadcast_to((128, CH)))

            # ================= attention =================
            with (
                tc.tile_pool(name="wrk", bufs=4) as wrk,
                tc.tile_pool(name="wrk2", bufs=4) as wrk2,
                tc.tile_pool(name="ttp", bufs=4) as ttp,
                tc.tile_pool(name="avp", bufs=2) as avp,
                tc.tile_pool(name="small", bufs=8) as small,
                tc.tile_pool(name="ps_m", bufs=3, space="PSUM") as ps_m,
                tc.tile_pool(name="ps_av", bufs=1, space="PSUM") as ps_av,
                tc.tile_pool(name="ps_avT", bufs=1, space="PSUM") as ps_avT,
            ):
                def phase_a_pair(hp, sb):
                    """QEr for heads (2hp, 2hp+1), q-block sb -> P[h] (fp8).

                    The two heads' matmuls go to different row groups
                    (base partitions 0/64) and stream concurrently.
                    Column 0 of every P row was pre-zeroed at startup, so
                    only the s QEr columns are written here."""
                    pex = [wrk.tile([128, s + 1], FP8, tag=f"pexp{i}",
                                    name=f"pexp{i}") for i in range(2)]
                    # P's zero column travels with the tile (at the END of
                    # the row -- the whole flat layout is shifted by one
                    # element, keeping the psum drains byte-aligned and the
                    # DRAM write fully contiguous)
                    nc.vector.memset(pex[0][:, s:s + 1], 0.0)
                    nc.vector.memset(pex[1][:, s:s + 1], 0.0)
                    for lt in range(NLT):
                        psA = ps_m.tile([128, LT], F32, tag="pm", name="psA")
                        psB = ps_m.tile([128, LT], F32, tag="pm", name="psB")
                        for c in range(LT // 512):
                            l0 = lt * LT + c * 512
                            nc.tensor.matmul(
                                psA[:, c * 512:(c + 1) * 512],
                                qT[hp][0:64, sb * 128:(sb + 1) * 128],
                                erT[0:64, l0:l0 + 512],
                                start=True, stop=True)
                            nc.tensor.matmul(
                                psB[:, c * 512:(c + 1) * 512],
                                qT[hp][64:128, sb * 128:(sb + 1) * 128],
                                erT[64:128, l0:l0 + 512],
                                start=True, stop=True)
                        # drains: DVE for head A; head B alternates DVE/ACT
                        # (GPSIMD cannot read PSUM on TRN2)
                        nc.vector.tensor_copy(
                            pex[0][:, lt * LT:(lt + 1) * LT], psA[:])
                        engb = nc.vector.tensor_copy if sb % 2 == 0 else \
                            nc.scalar.copy
                        engb(pex[1][:, lt * LT:(lt + 1) * LT], psB[:])
                    for i in range(2):
                        h = 2 * hp + i
                        base1 = sb * 128 * (s + 1)
                        nc.sync.dma_start(
                            p_ds[h][base1:base1 + 128 * (s + 1)]
                            .rearrange("(r c) -> r c", c=s + 1),
                            pex[i][:])

                def skew_read(h, sb):
                    """Skewed strided read of P[h] for q-block sb (the -1
                    accounts for the zero column sitting at the end of each
                    P row instead of the front)."""
                    st = wrk2.tile([128, s], FP8, tag="srel", name="srel",
                                   bufs=22)
                    base = (sb * 128 + 1) * s - 1
                    nc.sync.dma_start(
                        st[:],
                        p_ds[h][base:base + 128 * s]
                        .rearrange("(r c) -> r c", c=s))
                    return st

                def tp_step(hp, ssb, tp, srels, ttA, ttB):
                    """Transposed scores for one pair of t-blocks, both
                    heads of pair hp.

                    scoresT[t-block, i] = k_tb^T q (kT-block stationary, q
                    moving at N=512); the heads alternate row groups so the
                    two QK matmuls overlap in the array.  The four srel
                    skew-blocks are transpose-accumulated into the same PSUM
                    group via fp8 identity matmuls; exp drains PSUM straight
                    into the AV-ready ttile slices, [128,1024] per ACT
                    instruction."""
                    q0 = ssb * QG * 128
                    psA = ps_m.tile([128, 1024], F32, tag="pm", name="psA")
                    psB = ps_m.tile([128, 1024], F32, tag="pm", name="psB")
                    for j in range(2):
                        tb = 2 * tp + j
                        nc.tensor.matmul(
                            psA[:, j * 512:(j + 1) * 512],
                            kT[hp][0:64, tb * 128:(tb + 1) * 128],
                            qT[hp][0:64, q0:q0 + 512],
                            start=True, stop=False, skip_group_check=True)
                        nc.tensor.matmul(
                            psB[:, j * 512:(j + 1) * 512],
                            kT[hp][64:128, tb * 128:(tb + 1) * 128],
                            qT[hp][64:128, q0:q0 + 512],
                            start=True, stop=False, skip_group_check=True)
                    for j in range(2):
                        tb = 2 * tp + j
                        for ps, sr in ((psA, srels[0]), (psB, srels[1])):
                            for ib in range(QG):
                                nc.tensor.matmul(
                                    ps[:, j * 512 + ib * 128:
                                       j * 512 + (ib + 1) * 128],
                                    sr[ib][:, tb * 128:(tb + 1) * 128],
                                    ident_fp8[:],
                                    start=False, stop=(ib == QG - 1),
                                    skip_group_check=True)
                    nc.scalar.activation(
                        ttA[:, 2 * tp:2 * tp + 2, :]
                        .rearrange("p a c -> p (a c)"),
                        psA[:],
                        mybir.ActivationFunctionType.Exp, scale=SCALE)
                    nc.scalar.activation(
                        ttB[:, 2 * tp:2 * tp + 2, :]
                        .rearrange("p a c -> p (a c)"),
                        psB[:],
                        mybir.ActivationFunctionType.Exp, scale=SCALE)

                def phase_b_av(hloc, ssb, ttile):
                    """AV burst for one 512-query group; stash raw result."""
                    av_ps = ps_av.tile([HD + 1, 512], F32, tag="av",
                                       name="av_ps")
                    for ci in range(SB):
                        nc.tensor.matmul(
                            av_ps[:],
                            vaug[ci][:, hloc * (HD + 1):
                                     (hloc + 1) * (HD + 1)],
                            ttile[:, ci, :],
                            start=(ci == 0), stop=(ci == SB - 1))
                    avs = avp.tile([HD + 1, 512], BF16, tag="avs",
                                   name="avs")
                    nc.scalar.copy(avs[:], av_ps[:])
                    for q4 in range(QG):
                        sb = ssb * QG + q4
                        avT = ps_avT.tile([128, HD + 1], F32, tag="avT",
                                          name="avT")
                        nc.tensor.matmul(
                            avT[:], avs[:, q4 * 128:(q4 + 1) * 128],
                            ident_bf16[0:HD + 1, 0:HD + 1],
                            start=True, stop=True)
                        nc.vector.tensor_copy(
                            outp[sb][:, hloc * (HD + 1):
                                     (hloc + 1) * (HD + 1)],
                            avT[:])

                def div_block(sb):
                    """One batched reciprocal of the 4 heads' softmax sums
                    (at the 65-col slot tails), then scale the raw AV
                    numerators in place."""
                    o3 = outp[sb][:].rearrange("p (h c) -> p h c", c=HD + 1)
                    rinv4 = small.tile([128, HLOC], F32, tag="rinv4",
                                       name="rinv4")
                    nc.vector.reciprocal(rinv4[:], o3[:, :, HD])
                    for hh in range(HLOC):
                        nc.vector.tensor_scalar_mul(
                            o3[:, hh, 0:HD],
                            o3[:, hh, 0:HD],
                            rinv4[:, hh:hh + 1])

                def ln_stats_block(sb):
                    o3 = outp[sb][:].rearrange("p (h c) -> p h c", c=HD + 1)
                    s1 = small.tile([128, 1], F32, tag="s1", name="s1")
                    nc.vector.reduce_sum(
                        s1[:], o3[:, :, 0:HD], axis=mybir.AxisListType.XY)
                    sq = small.tile([128, 1], F32, tag="sq", name="sq")
                    scr = wrk.tile([128, CH], F32, tag="scr", name="scr")
                    nc.scalar.activation(
                        scr[:], o3[:, :, 0:HD],
                        mybir.ActivationFunctionType.Square, accum_out=sq[:])
                    nc.sync.dma_start(
                        cc_in[sb * 128:(sb + 1) * 128, 0:1], s1[:])
                    nc.sync.dma_start(
                        cc_in[sb * 128:(sb + 1) * 128, 1:2], sq[:])

                def ln_chunk_collective(ssb):
                    r0 = ssb * QG * 128
                    r1 = (ssb + 1) * QG * 128
                    nc.gpsimd.collective_compute(
                        "AllReduce", mybir.AluOpType.add,
                        replica_groups=pairs,
                        ins=[cc_in[r0:r1, :].opt()],
                        outs=[cc_out[r0:r1, :].opt()])

                def ln_apply_block(sb):
                    o3 = outp[sb][:].rearrange("p (h c) -> p h c", c=HD + 1)
                    st = small.tile([128, 2], F32, tag="st")
                    # cc_out read + final write go via the (idle) GPSIMD
                    # DMA queue: on the SP queue they'd block later skew
                    # reads behind the collective's latency
                    nc.gpsimd.dma_start(
                        st[:], cc_out[sb * 128:(sb + 1) * 128, :])
                    me2 = small.tile([128, 2], F32, tag="me2")
                    nc.vector.tensor_scalar_mul(me2[:], st[:], 1.0 / E)
                    msq = small.tile([128, 1], F32, tag="msq")
                    nc.vector.tensor_mul(msq[:], me2[:, 0:1], me2[:, 0:1])
                    var = small.tile([128, 1], F32, tag="var")
                    nc.vector.tensor_scalar(
                        var[:], me2[:, 1:2], msq[:], EPS,
                        op0=mybir.AluOpType.subtract,
                        op1=mybir.AluOpType.add)
                    vrec = small.tile([128, 1], F32, tag="vrec")
                    nc.vector.reciprocal(vrec[:], var[:])
                    rstd = small.tile([128, 1], F32, tag="rstd")
                    nc.scalar.activation(
                        rstd[:], vrec[:],
                        mybir.ActivationFunctionType.Sqrt)
                    tmp = wrk.tile([128, CH], F32, tag="tmp")
                    nc.vector.tensor_scalar(
                        tmp[:].rearrange("p (h c) -> p h c", c=HD),
                        o3[:, :, 0:HD], me2[:, 0:1], rstd[:],
                        op0=mybir.AluOpType.subtract,
                        op1=mybir.AluOpType.mult)
                    y1 = wrk2.tile([128, CH], F32, tag="y1")
                    nc.gpsimd.tensor_mul(y1[:], tmp[:], gamma_bc[:])
                    y2 = wrk.tile([128, CH], F32, tag="y2")
                    nc.gpsimd.tensor_add(y2[:], y1[:], beta_bc[:])
                    nc.gpsimd.dma_start(
                        out_d[sb * 128:(sb + 1) * 128, :], y2[:])

                # -------- main attention loop: ssb outer, heads inner ------
                # Software pipeline: the next group's QEr (phase A, PE-light
                # + DVE/ACT drains) is interleaved between this group's
                # score tp-steps (PE-dense, ACT exp drains) so every engine
                # sees work from two streams at all times.  AV bursts for
                # pair 0 run inside pair 1's score loop.
                NTP = SB // 2
                for hp in range(2):
                    for sb4 in range(QG):
                        phase_a_pair(hp, sb4)
                # early skew reads for heads 0/1 of the first group
                nxt_early = {h: [skew_read(h, sb4) for sb4 in range(QG - 1)]
                             for h in range(2)}
                for ssb in range(NSSB):
                    nxt = ssb + 1 < NSSB
                    nb = (ssb + 1) * QG
                    srels = nxt_early
                    # heads 2/3's early reads: their tp-steps run late
                    # enough that issuing here still hides the DMA
                    for h in range(2, HLOC):
                        srels[h] = [skew_read(h, ssb * QG + sb4)
                                    for sb4 in range(QG - 1)]
                    tts = {}
                    for hp in range(2):
                        if nxt:
                            phase_a_pair(hp, nb)
                        # last skew read: needs P row nb*128 (written above)
                        for i in range(2):
                            srels[2 * hp + i].append(
                                skew_read(2 * hp + i, ssb * QG + QG - 1))
                        ttA = ttp.tile([128, SB, 512], BF16, tag="tt",
                                       name="ttA")
                        ttB = ttp.tile([128, SB, 512], BF16, tag="tt",
                                       name="ttB")
                        tts[hp] = (ttA, ttB)
                        pend_a = list(range(nb + 1, nb + QG)) if nxt else []
                        av_done = [False, False]
                        for tp in range(NTP):
                            tp_step(hp, ssb,  tp,
                                    (srels[2 * hp], srels[2 * hp + 1]),
                                    ttA, ttB)
                            if tp % 2 == 1 and pend_a:
                                phase_a_pair(hp, pend_a.pop(0))
                            if hp == 1 and tp == 2:
                                phase_b_av(0, ssb, tts[0][0])
                                av_done[0] = True
                            if hp == 1 and tp == 5:
                                phase_b_av(1, ssb, tts[0][1])
                                av_done[1] = True
                        while pend_a:
                            phase_a_pair(hp, pend_a.pop(0))
                        if hp == 0:
                            # previous chunk's LN here: by now its
                            # AllReduce has had half an iteration to land,
                            # so the DVE queue isn't blocked at its head
                            if ssb > 0:
                                for sb4 in range(QG):
                                    ln_apply_block((ssb - 1) * QG + sb4)
                        else:
                            if not av_done[0]:
                                phase_b_av(0, ssb, tts[0][0])
                            if not av_done[1]:
                                phase_b_av(1, ssb, tts[0][1])
                    phase_b_av(2, ssb, tts[1][0])
                    phase_b_av(3, ssb, tts[1][1])
                    for sb4 in range(QG):
                        div_block(ssb * QG + sb4)
                        ln_stats_block(ssb * QG + sb4)
                    ln_chunk_collective(ssb)
                    # prefetch heads 0/1's skew reads for the next group
                    # (their P rows were written during this iteration)
                    nxt_early = {}
                    if nxt:
                        nxt_early = {
                            h: [skew_read(h, nb + sb4)
                                for sb4 in range(QG - 1)]
                            for h in range(2)}
                for sb4 in range(QG):
                    ln_apply_block((NSSB - 1) * QG + sb4)

    if legalize:
        _legalize_waits(nc)
    return nc


def _legalize_waits(nc):
    """walrus's codegen accepts at most one sync wait on most instruction
    structs; hoist extra waits onto NoOps inserted just before, on the
    same engine queue (program order preserves the semantics)."""
    n = 0
    keep = set()
    for bb in nc.main_func.blocks:
        out = []
        for inst in bb.instructions:
            si = inst.sync_info
            if (inst.opcode not in keep and si is not None
                    and si.on_wait and len(si.on_wait) > 1):
                for w in si.on_wait[:-1]:
                    nop = mybir.InstNoOp(
                        name=f"I-mmw{n}", ins=[], outs=[])
                    n += 1
                    nop.engine = inst.engine
                    nop.sync_info = mybir.SyncInfo(
                        on_wait=[w], on_update=[])
                    out.append(nop)
                si.on_wait = [si.on_wait[-1]]
            out.append(inst)
        bb.instructions = out


_NC_CACHE = {}


def _get_nc(s=S, n_cores=N_CORES):
    key = (s, n_cores)
    if key not in _NC_CACHE:
        _NC_CACHE[key] = build_nc(s, n_cores)
    return _NC_CACHE[key]


def make_in_maps(x, Wq, Wk, Wv, Er, gamma, beta, n_cores=N_CORES):
    in_maps = []
    for c in range(n_cores):
        b, hg = c // 2, c % 2
        sl = slice(hg * CH, (hg + 1) * CH)
        in_maps.append({
            "x": np.ascontiguousarray(x[b], dtype=np.float32),
            "wq": np.ascontiguousarray(Wq[sl], dtype=np.float32),
            "wk": np.ascontiguousarray(Wk[sl], dtype=np.float32),
            "wv": np.ascontiguousarray(Wv[sl], dtype=np.float32),
            "er": np.ascontiguousarray(Er, dtype=np.float32),
            "gamma": np.ascontiguousarray(gamma[sl], dtype=np.float32)[None, :],
            "beta": np.ascontiguousarray(beta[sl], dtype=np.float32)[None, :],
        })
    return in_maps


def assemble(results, n_cores=N_CORES, s=S):
    y = np.empty((n_cores // 2, s, E), np.float32)
    for c in range(n_cores):
        y[c // 2, :, (c % 2) * CH:(c % 2 + 1) * CH] = results[c]["out"]
    return y


def kernel(**inputs):
    from concourse.bass_utils import run_bass_kernel_spmd
    nc = _get_nc()
    in_maps = make_in_maps(
        inputs["x"], inputs["Wq"], inputs["Wk"], inputs["Wv"],
        inputs["Er"], inputs["gamma"], inputs["beta"])
    res = run_bass_kernel_spmd(nc, in_maps, list(range(N_CORES)))
    return assemble(res.results)


# revision 31
# speedup vs baseline: 1.2143x; 1.0604x over previous
"""Relative-position attention (Music-Transformer style skew) + LayerNorm,
distributed over 8 TRN2 NeuronCores.

Sharding: data-parallel over batch (B=4) x tensor-parallel over head-halves
(H=8 -> 2 groups of 4). Core c handles batch b=c//2, heads [4*(c%2), 4*(c%2)+4),
producing output channels [256*(c%2), +256) of y[b]. The final LayerNorm needs
full-E stats, exchanged via a tiny pairwise AllReduce of (sum, sumsq) per
512-row chunk, overlapped with the next chunk's compute.

Skew trick: Srel[i,j] = F[(i+1)*S + j] where F is the row-major flat view of
the padded matrix P[i, 0]=0, P[i, 1+l]=QEr[i, l] (P is [S, S+1]). We bounce P
through DRAM in fp8-e4m3; the skewed read back is a plain strided DMA.

PE strategy: everything on the PE is bf16/fp8 (enables FWL weight loads and
1 cycle/row at every clock state; fp32 runs multi-pass).  QEr and QK have
contraction dim 64 (head dim), so the two heads of a 128-partition tile are
issued back-to-back to different row groups (base partitions 0/64) and stream
through the array concurrently (~2x).  Srel is injected into the transposed
score PSUM via fp8 identity matmuls (fused transpose + add).  AV uses flipped
operand roles: the V block (65 cols incl. a ones column for the softmax
denominator) is PE-stationary and the exp'd transposed scores stream at N=512.
"""

import numpy as np

import concourse.bass as bass
import concourse.mybir as mybir
from concourse import masks
from concourse.tile import TileContext

F32 = mybir.dt.float32
BF16 = mybir.dt.bfloat16
FP8 = mybir.dt.float8e4

B, S, E, H = 4, 2048, 512, 8
HD = E // H          # 64
HLOC = 4             # heads per core
CH = HLOC * HD       # 256 output channels per core
SCALE = float(E) ** -0.5
EPS = 1e-5
N_CORES = 8
QG = 4               # 128-row q-blocks per 512-query group


def build_nc(s=S, n_cores=N_CORES, debug=False, legalize=True):
    """Build the per-core Bass graph (SPMD: same graph on all cores)."""
    nc = bass.Bass(target_bir_lowering=False, debug=debug)

    SB = s // 128        # number of 128-row blocks
    KC = s // 512        # number of 512-col chunks
    NSSB = SB // QG      # number of 512-query groups
    LT = min(1024, s)    # phase-A psum tile width (2 banks)
    NLT = s // LT

    x_d = nc.declare_dram_parameter("x", [s, E], F32, isOutput=False)
    wq_d = nc.declare_dram_parameter("wq", [CH, E], F32, isOutput=False)
    wk_d = nc.declare_dram_parameter("wk", [CH, E], F32, isOutput=False)
    wv_d = nc.declare_dram_parameter("wv", [CH, E], F32, isOutput=False)
    er_d = nc.declare_dram_parameter("er", [s, HD], F32, isOutput=False)
    gamma_d = nc.declare_dram_parameter("gamma", [1, CH], F32, isOutput=False)
    beta_d = nc.declare_dram_parameter("beta", [1, CH], F32, isOutput=False)
    out_d = nc.declare_dram_parameter("out", [s, CH], F32, isOutput=True)

    # Padded-QEr bounce buffers, one per head, flat [S*(S+1)] fp8.
    p_ds = [nc.dram_tensor(f"pbuf{h}", [s * (s + 1)], FP8)
            for h in range(HLOC)]
    cc_in = nc.dram_tensor("cc_in", [s, 2], F32)
    cc_out = nc.dram_tensor("cc_out", [s, 2], F32)

    pairs = [[2 * i, 2 * i + 1] for i in range(n_cores // 2)]

    with TileContext(nc) as tc:
        with (
            tc.tile_pool(name="const", bufs=1) as const_pool,
            tc.tile_pool(name="persist", bufs=1) as pp,
        ):
            ident_bf16 = const_pool.tile([128, 128], BF16)
            ident_fp8 = const_pool.tile([128, 128], FP8)
            masks.make_identity(nc, ident_bf16[:])
            masks.make_identity(nc, ident_fp8[:])
            gamma_bc = const_pool.tile([128, CH], F32)
            beta_bc = const_pool.tile([128, CH], F32)

            # ---- persistent SBUF tensors (all bf16 on the PE paths) ----
            # ErT replicated into both partition halves so each head of a
            # pair finds it at its own base partition.
            erT = pp.tile([128, s], BF16, tag="erT")
            qT = [pp.tile([128, s], BF16, tag=f"qT{oc}", name=f"qT{oc}")
                  for oc in range(2)]
            kT = [pp.tile([128, s], BF16, tag=f"kT{oc}", name=f"kT{oc}")
                  for oc in range(2)]
            # v with a ones column appended per head: [128, HLOC*(HD+1)] bf16
            vaug = [pp.tile([128, HLOC * (HD + 1)], BF16, tag=f"va{sb}",
                            name=f"va{sb}") for sb in range(SB)]
            # raw AV numerators + softmax sum per head (65-col head slots,
            # matching the avT transpose output so it lands in one copy)
            outp = [pp.tile([128, HLOC * (HD + 1)], F32, tag=f"op{sb}",
                            name=f"op{sb}") for sb in range(SB)]

            # Long-lived attention pools open first so short-lived setup
            # pools (xw, ld, cst, ps_set, ps_pj) can close in LIFO order.
            from contextlib import ExitStack
            att_stk = ExitStack()
            wrk = att_stk.enter_context(tc.tile_pool(name="wrk", bufs=4))
            ps_m = att_stk.enter_context(
                tc.tile_pool(name="ps_m", bufs=3, space="PSUM"))

            # ====== setup + projections (xT/wT freed afterwards) ======
            xw_pool = tc.tile_pool(name="xw", bufs=1)
            xwp = xw_pool.__enter__()
            xT = [xwp.tile([128, s], BF16, tag=f"xT{ec}", name=f"xT{ec}")
                  for ec in range(4)]
            wT = {
                w: [xwp.tile([128, CH], BF16, tag=f"{w}T{ec}",
                             name=f"{w}T{ec}") for ec in range(4)]
                for w in ("wq", "wk", "wv")
            }
            with (
                tc.tile_pool(name="ld", bufs=4) as ld_pool,
                tc.tile_pool(name="cst", bufs=4) as cst_pool,
                tc.tile_pool(name="ps_set", bufs=2, space="PSUM") as ps_set,
            ):
                # Warm-up: absorb the Pool (identity-creation) dependency
                # into PE's observed clock.
                warm = ps_set.tile([128, 128], F32, tag="pset")
                nc.tensor.matmul(
                    warm[:], ident_bf16[:], ident_bf16[:],
                    start=True, stop=True)

                # xT[ec][:, i*128:(i+1)*128] = bf16(x[i-block, ec-block]).T
                for sb in range(SB):
                    xt = ld_pool.tile([128, E], F32, tag="xld")
                    nc.sync.dma_start(xt[:], x_d[sb * 128:(sb + 1) * 128, :])
                    xb = cst_pool.tile([128, E], BF16, tag="xcst")
                    nc.vector.tensor_copy(xb[:], xt[:])
                    for ec in range(4):
                        pst = ps_set.tile([128, 128], F32, tag="pset")
                        nc.tensor.matmul(
                            pst[:], xb[:, ec * 128:(ec + 1) * 128],
                            ident_bf16[:], start=True, stop=True)
                        eng = nc.scalar.copy if ec % 2 else \
                            nc.vector.tensor_copy
                        eng(xT[ec][:, sb * 128:(sb + 1) * 128], pst[:])
                # weights
                for w_name, w_d in (("wq", wq_d), ("wk", wk_d), ("wv", wv_d)):
                    for pc in range(CH // 128):
                        wt = ld_pool.tile([128, E], F32, tag="wld")
                        nc.sync.dma_start(
                            wt[:], w_d[pc * 128:(pc + 1) * 128, :])
                        wb = cst_pool.tile([128, E], BF16, tag="wcst")
                        nc.vector.tensor_copy(wb[:], wt[:])
                        for ec in range(4):
                            pst = ps_set.tile([128, 128], F32, tag="pset")
                            nc.tensor.matmul(
                                pst[:], wb[:, ec * 128:(ec + 1) * 128],
                                ident_bf16[:], start=True, stop=True)
                            eng = nc.scalar.copy if ec % 2 else \
                                nc.vector.tensor_copy
                            eng(wT[w_name][ec][:, pc * 128:(pc + 1) * 128],
                                pst[:])
                # Er: one load+cast, then transpose into both partition halves
                et = ld_pool.tile([128, SB * HD], F32, tag="eld")
                nc.sync.dma_start(
                    et[:].rearrange("p (b d) -> p b d", d=HD),
                    er_d[:].rearrange("(b p) d -> p b d", p=128))
                eb = cst_pool.tile([128, SB * HD], BF16, tag="ecst")
                nc.vector.tensor_copy(eb[:], et[:])
                for sb in range(SB):
                    pst = ps_set.tile([128, 128], F32, tag="pset")
                    nc.tensor.matmul(
                        pst[0:64, :], eb[:, sb * HD:(sb + 1) * HD],
                        ident_bf16[:], start=True, stop=True)
                    nc.tensor.matmul(
                        pst[64:128, :], eb[:, sb * HD:(sb + 1) * HD],
                        ident_bf16[:], start=True, stop=True)
                    nc.vector.tensor_copy(
                        erT[:, sb * 128:(sb + 1) * 128], pst[:])

            # ================= attention =================
            if True:
                def phase_a_pair(hp, sb, first=False):
                    """QEr for heads (2hp, 2hp+1), q-block sb -> P[h] (fp8).

                    The two heads' matmuls go to different row groups
                    (base partitions 0/64) and stream concurrently."""
                    pex = [wrk.tile([128, s + 1], FP8, tag=f"pexp{i}",
                                    name=f"pexp{i}") for i in range(2)]
                    # P's zero column travels with the tile (at the END of
                    # the row -- the whole flat layout is shifted by one
                    # element, keeping the psum drains byte-aligned and the
                    # DRAM write fully contiguous).  The drains never touch
                    # col s, so zeroing each ring buffer once (during the
                    # prologue, which cycles every buf) is enough.
                    if first:
                        nc.vector.memset(pex[0][:, s:s + 1], 0.0)
                        nc.vector.memset(pex[1][:, s:s + 1], 0.0)
                    for lt in range(NLT):
                        psA = ps_m.tile([128, LT], F32, tag="pm", name="psA")
                        psB = ps_m.tile([128, LT], F32, tag="pm", name="psB")
                        for c in range(LT // 512):
                            l0 = lt * LT + c * 512
                            nc.tensor.matmul(
                                psA[:, c * 512:(c + 1) * 512],
                                qT[hp][0:64, sb * 128:(sb + 1) * 128],
                                erT[0:64, l0:l0 + 512],
                                start=True, stop=True)
                            nc.tensor.matmul(
                                psB[:, c * 512:(c + 1) * 512],
                                qT[hp][64:128, sb * 128:(sb + 1) * 128],
                                erT[64:128, l0:l0 + 512],
                                start=True, stop=True)
                        # drains: DVE for head A; head B alternates DVE/ACT
                        # (GPSIMD cannot read PSUM on TRN2)
                        nc.vector.tensor_copy(
                            pex[0][:, lt * LT:(lt + 1) * LT], psA[:])
                        engb = nc.vector.tensor_copy if sb % 2 == 0 else \
                            nc.scalar.copy
                        engb(pex[1][:, lt * LT:(lt + 1) * LT], psB[:])
                    for i in range(2):
                        h = 2 * hp + i
                        base1 = sb * 128 * (s + 1)
                        nc.sync.dma_start(
                            p_ds[h][base1:base1 + 128 * (s + 1)]
                            .rearrange("(r c) -> r c", c=s + 1),
                            pex[i][:])

                def skew_read(h, sb):
                    """Skewed strided read of P[h] for q-block sb (the -1
                    accounts for the zero column sitting at the end of each
                    P row instead of the front)."""
                    st = wrk2.tile([128, s], FP8, tag="srel", name="srel",
                                   bufs=22)
                    base = (sb * 128 + 1) * s - 1
                    nc.sync.dma_start(
                        st[:],
                        p_ds[h][base:base + 128 * s]
                        .rearrange("(r c) -> r c", c=s))
                    return st

                def tp_step(hp, ssb, tp, srels, ttA, ttB):
                    """Transposed scores for one pair of t-blocks, both
                    heads of pair hp.

                    scoresT[t-block, i] = k_tb^T q (kT-block stationary, q
                    moving at N=512); the heads alternate row groups so the
                    two QK matmuls overlap in the array.  The four srel
                    skew-blocks are transpose-accumulated into the same PSUM
                    group via fp8 identity matmuls; exp drains PSUM straight
                    into the AV-ready ttile slices, [128,1024] per ACT
                    instruction."""
                    q0 = ssb * QG * 128
                    psA = ps_m.tile([128, 1024], F32, tag="pm", name="psA")
                    psB = ps_m.tile([128, 1024], F32, tag="pm", name="psB")
                    for j in range(2):
                        tb = 2 * tp + j
                        nc.tensor.matmul(
                            psA[:, j * 512:(j + 1) * 512],
                            kT[hp][0:64, tb * 128:(tb + 1) * 128],
                            qT[hp][0:64, q0:q0 + 512],
                            start=True, stop=False, skip_group_check=True)
                        nc.tensor.matmul(
                            psB[:, j * 512:(j + 1) * 512],
                            kT[hp][64:128, tb * 128:(tb + 1) * 128],
                            qT[hp][64:128, q0:q0 + 512],
                            start=True, stop=False, skip_group_check=True)
                    for j in range(2):
                        tb = 2 * tp + j
                        for ps, sr in ((psA, srels[0]), (psB, srels[1])):
                            for ib in range(QG):
                                nc.tensor.matmul(
                                    ps[:, j * 512 + ib * 128:
                                       j * 512 + (ib + 1) * 128],
                                    sr[ib][:, tb * 128:(tb + 1) * 128],
                                    ident_fp8[:],
                                    start=False, stop=(ib == QG - 1),
                                    skip_group_check=True)
                    nc.scalar.activation(
                        ttA[:, 2 * tp:2 * tp + 2, :]
                        .rearrange("p a c -> p (a c)"),
                        psA[:],
                        mybir.ActivationFunctionType.Exp, scale=SCALE)
                    nc.scalar.activation(
                        ttB[:, 2 * tp:2 * tp + 2, :]
                        .rearrange("p a c -> p (a c)"),
                        psB[:],
                        mybir.ActivationFunctionType.Exp, scale=SCALE)

                def phase_b_av(hloc, ssb, ttile):
                    """AV burst for one 512-query group; stash raw result."""
                    av_ps = ps_av.tile([HD + 1, 512], F32, tag="av",
                                       name="av_ps")
                    for ci in range(SB):
                        nc.tensor.matmul(
                            av_ps[:],
                            vaug[ci][:, hloc * (HD + 1):
                                     (hloc + 1) * (HD + 1)],
                            ttile[:, ci, :],
                            start=(ci == 0), stop=(ci == SB - 1))
                    avs = avp.tile([HD + 1, 512], BF16, tag="avs",
                                   name="avs")
                    nc.scalar.copy(avs[:], av_ps[:])
                    for q4 in range(QG):
                        sb = ssb * QG + q4
                        avT = ps_avT.tile([128, HD + 1], F32, tag="avT",
                                          name="avT")
                        nc.tensor.matmul(
                            avT[:], avs[:, q4 * 128:(q4 + 1) * 128],
                            ident_bf16[0:HD + 1, 0:HD + 1],
                            start=True, stop=True)
                        nc.vector.tensor_copy(
                            outp[sb][:, hloc * (HD + 1):
                                     (hloc + 1) * (HD + 1)],
                            avT[:])

                def div_block(sb):
                    """One batched reciprocal of the 4 heads' softmax sums
                    (at the 65-col slot tails), then scale the raw AV
                    numerators in place."""
                    o3 = outp[sb][:].rearrange("p (h c) -> p h c", c=HD + 1)
                    rinv4 = small.tile([128, HLOC], F32, tag="rinv4",
                                       name="rinv4")
                    nc.vector.reciprocal(rinv4[:], o3[:, :, HD])
                    for hh in range(HLOC):
                        nc.vector.tensor_scalar_mul(
                            o3[:, hh, 0:HD],
                            o3[:, hh, 0:HD],
                            rinv4[:, hh:hh + 1])

                def ln_stats_block(sb):
                    o3 = outp[sb][:].rearrange("p (h c) -> p h c", c=HD + 1)
                    s1 = small.tile([128, 1], F32, tag="s1", name="s1")
                    nc.vector.reduce_sum(
                        s1[:], o3[:, :, 0:HD], axis=mybir.AxisListType.XY)
                    sq = small.tile([128, 1], F32, tag="sq", name="sq")
                    scr = wrk.tile([128, CH], F32, tag="scr", name="scr")
                    nc.scalar.activation(
                        scr[:], o3[:, :, 0:HD],
                        mybir.ActivationFunctionType.Square, accum_out=sq[:])
                    nc.sync.dma_start(
                        cc_in[sb * 128:(sb + 1) * 128, 0:1], s1[:])
                    nc.sync.dma_start(
                        cc_in[sb * 128:(sb + 1) * 128, 1:2], sq[:])

                def ln_chunk_collective(ssb):
                    r0 = ssb * QG * 128
                    r1 = (ssb + 1) * QG * 128
                    nc.gpsimd.collective_compute(
                        "AllReduce", mybir.AluOpType.add,
                        replica_groups=pairs,
                        ins=[cc_in[r0:r1, :].opt()],
                        outs=[cc_out[r0:r1, :].opt()])

                def ln_apply_block(sb):
                    o3 = outp[sb][:].rearrange("p (h c) -> p h c", c=HD + 1)
                    st = small.tile([128, 2], F32, tag="st")
                    # cc_out read + final write go via the (idle) GPSIMD
                    # DMA queue: on the SP queue they'd block later skew
                    # reads behind the collective's latency
                    nc.gpsimd.dma_start(
                        st[:], cc_out[sb * 128:(sb + 1) * 128, :])
                    me2 = small.tile([128, 2], F32, tag="me2")
                    nc.vector.tensor_scalar_mul(me2[:], st[:], 1.0 / E)
                    msq = small.tile([128, 1], F32, tag="msq")
                    nc.vector.tensor_mul(msq[:], me2[:, 0:1], me2[:, 0:1])
                    var = small.tile([128, 1], F32, tag="var")
                    nc.vector.tensor_scalar(
                        var[:], me2[:, 1:2], msq[:], EPS,
                        op0=mybir.AluOpType.subtract,
                        op1=mybir.AluOpType.add)
                    vrec = small.tile([128, 1], F32, tag="vrec")
                    nc.vector.reciprocal(vrec[:], var[:])
                    rstd = small.tile([128, 1], F32, tag="rstd")
                    nc.scalar.activation(
                        rstd[:], vrec[:],
                        mybir.ActivationFunctionType.Sqrt)
                    tmp = wrk.tile([128, CH], F32, tag="tmp")
                    nc.vector.tensor_scalar(
                        tmp[:].rearrange("p (h c) -> p h c", c=HD),
                        o3[:, :, 0:HD], me2[:, 0:1], rstd[:],
                        op0=mybir.AluOpType.subtract,
                        op1=mybir.AluOpType.mult)
                    y1 = wrk2.tile([128, CH], F32, tag="y1")
                    nc.gpsimd.tensor_mul(y1[:], tmp[:], gamma_bc[:])
                    y2 = wrk.tile([128, CH], F32, tag="y2")
                    nc.gpsimd.tensor_add(y2[:], y1[:], beta_bc[:])
                    nc.gpsimd.dma_start(
                        out_d[sb * 128:(sb + 1) * 128, :], y2[:])

                # ====== projections, interleaved with the QEr prologue =====
                # q first (feeds phase A), then the first group's QEr pairs
                # woven between the k/v projection matmuls so the PE stream
                # stays dense while the QEr drains + DMA round trip complete.
                ps_pj = tc.tile_pool(name="ps_pj", bufs=2, space="PSUM")
                pjp = ps_pj.__enter__()

                def proj_qk(dst, w_name, oc, scs):
                    for sc in scs:
                        ps = pjp.tile([128, 512], F32, tag="pj", name="pj")
                        for ec in range(4):
                            nc.tensor.matmul(
                                ps[:],
                                wT[w_name][ec][:, oc * 128:(oc + 1) * 128],
                                xT[ec][:, sc * 512:(sc + 1) * 512],
                                start=(ec == 0), stop=(ec == 3))
                        eng = nc.scalar.copy if sc % 2 else \
                            nc.vector.tensor_copy
                        eng(dst[oc][:, sc * 512:(sc + 1) * 512], ps[:])

                def proj_v(sbs):
                    for sb in sbs:
                        ps = pjp.tile([128, CH], F32, tag="pj", name="pj")
                        for ec in range(4):
                            nc.tensor.matmul(
                                ps[:],
                                xT[ec][:, sb * 128:(sb + 1) * 128],
                                wT["wv"][ec][:],
                                start=(ec == 0), stop=(ec == 3))
                        for h in range(HLOC):
                            nc.scalar.copy(
                                vaug[sb][:, h * (HD + 1):h * (HD + 1) + HD],
                                ps[:, h * HD:(h + 1) * HD])
                            nc.vector.memset(
                                vaug[sb][:, h * (HD + 1) + HD:
                                         (h + 1) * (HD + 1)],
                                1.0)

                for oc in range(2):
                    proj_qk(qT, "wq", oc, range(KC))
                pend0 = list(range(QG))
                for oc in range(2):
                    for scp in range(0, KC, 2):
                        if pend0:
                            phase_a_pair(0, pend0.pop(0), first=True)
                        proj_qk(kT, "wk", oc, range(scp, min(scp + 2, KC)))
                while pend0:
                    phase_a_pair(0, pend0.pop(0), first=True)
                for vb in range(QG):
                    phase_a_pair(1, vb, first=True)
                    proj_v(range(vb * SB // QG, (vb + 1) * SB // QG))
                ps_pj.__exit__(None, None, None)
                xw_pool.__exit__(None, None, None)
                # LN constants, needed only at the tail of each chunk
                nc.sync.dma_start(gamma_bc[:],
                                  gamma_d[:].broadcast_to((128, CH)))
                nc.sync.dma_start(beta_bc[:],
                                  beta_d[:].broadcast_to((128, CH)))
                wrk2 = att_stk.enter_context(
                    tc.tile_pool(name="wrk2", bufs=4))
                ttp = att_stk.enter_context(tc.tile_pool(name="ttp", bufs=4))
                avp = att_stk.enter_context(tc.tile_pool(name="avp", bufs=2))
                small = att_stk.enter_context(
                    tc.tile_pool(name="small", bufs=8))
                ps_av = att_stk.enter_context(
                    tc.tile_pool(name="ps_av", bufs=1, space="PSUM"))
                ps_avT = att_stk.enter_context(
                    tc.tile_pool(name="ps_avT", bufs=1, space="PSUM"))

                # -------- main attention loop: ssb outer, heads inner ------
                # Software pipeline: the next group's QEr (phase A, PE-light
                # + DVE/ACT drains) is interleaved between this group's
                # score tp-steps (PE-dense, ACT exp drains) so every engine
                # sees work from two streams at all times.  AV bursts for
                # pair 0 run inside pair 1's score loop.
                NTP = SB // 2
                # early skew reads for heads 0/1 of the first group
                nxt_early = {h: [skew_read(h, sb4) for sb4 in range(QG - 1)]
                             for h in range(2)}
                for ssb in range(NSSB):
                    nxt = ssb + 1 < NSSB
                    nb = (ssb + 1) * QG
                    srels = nxt_early
                    # heads 2/3's early reads: their tp-steps run late
                    # enough that issuing here still hides the DMA
                    for h in range(2, HLOC):
                        srels[h] = [skew_read(h, ssb * QG + sb4)
                                    for sb4 in range(QG - 1)]
                    tts = {}
                    for hp in range(2):
                        if nxt:
                            phase_a_pair(hp, nb)
                        # last skew read: needs P row nb*128 (written above)
                        for i in range(2):
                            srels[2 * hp + i].append(
                                skew_read(2 * hp + i, ssb * QG + QG - 1))
                        ttA = ttp.tile([128, SB, 512], BF16, tag="tt",
                                       name="ttA")
                        ttB = ttp.tile([128, SB, 512], BF16, tag="tt",
                                       name="ttB")
                        tts[hp] = (ttA, ttB)
                        pend_a = list(range(nb + 1, nb + QG)) if nxt else []
                        av_done = [False, False]
                        for tp in range(NTP):
                            tp_step(hp, ssb,  tp,
                                    (srels[2 * hp], srels[2 * hp + 1]),
                                    ttA, ttB)
                            if tp % 2 == 1 and pend_a:
                                phase_a_pair(hp, pend_a.pop(0))
                            if hp == 1 and tp == 2:
                                phase_b_av(0, ssb, tts[0][0])
                                av_done[0] = True
                            if hp == 1 and tp == 5:
                                phase_b_av(1, ssb, tts[0][1])
                                av_done[1] = True
                        while pend_a:
                            phase_a_pair(hp, pend_a.pop(0))
                        if hp == 0:
                            # previous chunk's LN here: by now its
                            # AllReduce has had half an iteration to land,
                            # so the DVE queue isn't blocked at its head
                            if ssb > 0:
                                for sb4 in range(QG):
                                    ln_apply_block((ssb - 1) * QG + sb4)
                        else:
                            if not av_done[0]:
                                phase_b_av(0, ssb, tts[0][0])
                            if not av_done[1]:
                                phase_b_av(1, ssb, tts[0][1])
                    phase_b_av(2, ssb, tts[1][0])
                    phase_b_av(3, ssb, tts[1][1])
                    for sb4 in range(QG):
                        div_block(ssb * QG + sb4)
                        ln_stats_block(ssb * QG + sb4)
                    ln_chunk_collective(ssb)
                    # prefetch heads 0/1's skew reads for the next group
                    # (their P rows were written during this iteration)
                    nxt_early = {}
                    if nxt:
                        nxt_early = {
                            h: [skew_read(h, nb + sb4)
                                for sb4 in range(QG - 1)]
                            for h in range(2)}
                for sb4 in range(QG):
                    ln_apply_block((NSSB - 1) * QG + sb4)
                att_stk.close()

    if legalize:
        _legalize_waits(nc)
    return nc


def _legalize_waits(nc):
    """walrus's codegen accepts at most one sync wait on most instruction
    structs; hoist extra waits onto NoOps inserted just before, on the
    same engine queue (program order preserves the semantics)."""
    n = 0
    keep = set()
    for bb in nc.main_func.blocks:
        out = []
        for inst in bb.instructions:
            si = inst.sync_info
            if (inst.opcode not in keep and si is not None
                    and si.on_wait and len(si.on_wait) > 1):
                for w in si.on_wait[:-1]:
                    nop = mybir.InstNoOp(
                        name=f"I-mmw{n}", ins=[], outs=[])
                    n += 1
                    nop.engine = inst.engine
                    nop.sync_info = mybir.SyncInfo(
                        on_wait=[w], on_update=[])
                    out.append(nop)
                si.on_wait = [si.on_wait[-1]]
            out.append(inst)
        bb.instructions = out


_NC_CACHE = {}


def _get_nc(s=S, n_cores=N_CORES):
    key = (s, n_cores)
    if key not in _NC_CACHE:
        _NC_CACHE[key] = build_nc(s, n_cores)
    return _NC_CACHE[key]


def make_in_maps(x, Wq, Wk, Wv, Er, gamma, beta, n_cores=N_CORES):
    in_maps = []
    for c in range(n_cores):
        b, hg = c // 2, c % 2
        sl = slice(hg * CH, (hg + 1) * CH)
        in_maps.append({
            "x": np.ascontiguousarray(x[b], dtype=np.float32),
            "wq": np.ascontiguousarray(Wq[sl], dtype=np.float32),
            "wk": np.ascontiguousarray(Wk[sl], dtype=np.float32),
            "wv": np.ascontiguousarray(Wv[sl], dtype=np.float32),
            "er": np.ascontiguousarray(Er, dtype=np.float32),
            "gamma": np.ascontiguousarray(gamma[sl], dtype=np.float32)[None, :],
            "beta": np.ascontiguousarray(beta[sl], dtype=np.float32)[None, :],
        })
    return in_maps


def assemble(results, n_cores=N_CORES, s=S):
    y = np.empty((n_cores // 2, s, E), np.float32)
    for c in range(n_cores):
        y[c // 2, :, (c % 2) * CH:(c % 2 + 1) * CH] = results[c]["out"]
    return y


def kernel(**inputs):
    from concourse.bass_utils import run_bass_kernel_spmd
    nc = _get_nc()
    in_maps = make_in_maps(
        inputs["x"], inputs["Wq"], inputs["Wk"], inputs["Wv"],
        inputs["Er"], inputs["gamma"], inputs["beta"])
    res = run_bass_kernel_spmd(nc, in_maps, list(range(N_CORES)))
    return assemble(res.results)


# revision 35
# speedup vs baseline: 1.2856x; 1.0587x over previous
"""Relative-position attention (Music-Transformer style skew) + LayerNorm,
distributed over 8 TRN2 NeuronCores.

Sharding: data-parallel over batch (B=4) x tensor-parallel over head-halves
(H=8 -> 2 groups of 4). Core c handles batch b=c//2, heads [4*(c%2), 4*(c%2)+4),
producing output channels [256*(c%2), +256) of y[b]. The final LayerNorm needs
full-E stats, exchanged via a tiny pairwise AllReduce of (sum, sumsq) per
512-row chunk, overlapped with the next chunk's compute.

Skew trick: Srel[i,j] = F[(i+1)*S + j] where F is the row-major flat view of
the padded matrix P[i, 0]=0, P[i, 1+l]=QEr[i, l] (P is [S, S+1]). We bounce P
through DRAM in fp8-e4m3; the skewed read back is a plain strided DMA.

PE strategy: everything on the PE is bf16/fp8 (enables FWL weight loads and
1 cycle/row at every clock state; fp32 runs multi-pass).  QEr and QK have
contraction dim 64 (head dim), so the two heads of a 128-partition tile are
issued back-to-back to different row groups (base partitions 0/64) and stream
through the array concurrently (~2x).  Srel is injected into the transposed
score PSUM via fp8 identity matmuls (fused transpose + add).  AV uses flipped
operand roles: the V block (65 cols incl. a ones column for the softmax
denominator) is PE-stationary and the exp'd transposed scores stream at N=512.
"""

import numpy as np

import concourse.bass as bass
import concourse.mybir as mybir
from concourse import masks
from concourse.tile import TileContext

F32 = mybir.dt.float32
BF16 = mybir.dt.bfloat16
FP8 = mybir.dt.float8e4

B, S, E, H = 4, 2048, 512, 8
HD = E // H          # 64
HLOC = 4             # heads per core
CH = HLOC * HD       # 256 output channels per core
SCALE = float(E) ** -0.5
EPS = 1e-5
N_CORES = 8
QG = 4               # 128-row q-blocks per 512-query group


def build_nc(s=S, n_cores=N_CORES, debug=False, legalize=True):
    """Build the per-core Bass graph (SPMD: same graph on all cores)."""
    nc = bass.Bass(target_bir_lowering=False, debug=debug)

    SB = s // 128        # number of 128-row blocks
    KC = s // 512        # number of 512-col chunks
    NSSB = SB // QG      # number of 512-query groups
    LT = min(1024, s)    # phase-A psum tile width (2 banks)
    NLT = s // LT

    x_d = nc.declare_dram_parameter("x", [s, E], F32, isOutput=False)
    wq_d = nc.declare_dram_parameter("wq", [CH, E], F32, isOutput=False)
    wk_d = nc.declare_dram_parameter("wk", [CH, E], F32, isOutput=False)
    wv_d = nc.declare_dram_parameter("wv", [CH, E], F32, isOutput=False)
    er_d = nc.declare_dram_parameter("er", [s, HD], F32, isOutput=False)
    gamma_d = nc.declare_dram_parameter("gamma", [1, CH], F32, isOutput=False)
    beta_d = nc.declare_dram_parameter("beta", [1, CH], F32, isOutput=False)
    out_d = nc.declare_dram_parameter("out", [s, CH], F32, isOutput=True)

    # Padded-QEr bounce buffers, one per head, flat [S*(S+1)] fp8.
    p_ds = [nc.dram_tensor(f"pbuf{h}", [s * (s + 1)], FP8)
            for h in range(HLOC)]
    cc_in = nc.dram_tensor("cc_in", [s, 2], F32)
    cc_out = nc.dram_tensor("cc_out", [s, 2], F32)

    pairs = [[2 * i, 2 * i + 1] for i in range(n_cores // 2)]

    with TileContext(nc) as tc:
        with (
            tc.tile_pool(name="const", bufs=1) as const_pool,
            tc.tile_pool(name="persist", bufs=1) as pp,
        ):
            ident_bf16 = const_pool.tile([128, 128], BF16)
            ident_fp8 = const_pool.tile([128, 128], FP8)
            masks.make_identity(nc, ident_bf16[:])
            masks.make_identity(nc, ident_fp8[:])
            gamma_bc = const_pool.tile([128, CH], F32)
            beta_bc = const_pool.tile([128, CH], F32)

            # ---- persistent SBUF tensors (all bf16 on the PE paths) ----
            # ErT replicated into both partition halves so each head of a
            # pair finds it at its own base partition.
            erT = pp.tile([128, s], BF16, tag="erT")
            qT = [pp.tile([128, s], BF16, tag=f"qT{oc}", name=f"qT{oc}")
                  for oc in range(2)]
            kT = [pp.tile([128, s], BF16, tag=f"kT{oc}", name=f"kT{oc}")
                  for oc in range(2)]
            # v with a ones column appended per head: [128, HLOC*(HD+1)] bf16
            vaug = [pp.tile([128, HLOC * (HD + 1)], BF16, tag=f"va{sb}",
                            name=f"va{sb}") for sb in range(SB)]
            # raw AV numerators + softmax sum per head (65-col head slots,
            # matching the avT transpose output so it lands in one copy)
            outp = [pp.tile([128, HLOC * (HD + 1)], F32, tag=f"op{sb}",
                            name=f"op{sb}") for sb in range(SB)]

            # Long-lived attention pools open first so short-lived setup
            # pools (xw, ld, cst, ps_set, ps_pj) can close in LIFO order.
            from contextlib import ExitStack
            att_stk = ExitStack()
            wrk = att_stk.enter_context(tc.tile_pool(name="wrk", bufs=4))
            ps_m = att_stk.enter_context(
                tc.tile_pool(name="ps_m", bufs=3, space="PSUM"))

            # ====== setup + projections (xT/wT freed afterwards) ======
            xw_pool = tc.tile_pool(name="xw", bufs=1)
            xwp = xw_pool.__enter__()
            xT = [xwp.tile([128, s], BF16, tag=f"xT{ec}", name=f"xT{ec}")
                  for ec in range(4)]
            wT = {
                w: [xwp.tile([128, CH], BF16, tag=f"{w}T{ec}",
                             name=f"{w}T{ec}") for ec in range(4)]
                for w in ("wq", "wk", "wv")
            }
            with (
                tc.tile_pool(name="ld", bufs=6) as ld_pool,
                tc.tile_pool(name="cst", bufs=6) as cst_pool,
                tc.tile_pool(name="ps_set", bufs=2, space="PSUM") as ps_set,
            ):
                # Warm-up: absorb the Pool (identity-creation) dependency
                # into PE's observed clock.
                warm = ps_set.tile([128, 128], F32, tag="pset")
                nc.tensor.matmul(
                    warm[:], ident_bf16[:], ident_bf16[:],
                    start=True, stop=True)

                # xT[ec][:, i*128:(i+1)*128] = bf16(x[i-block, ec-block]).T
                for sb in range(SB):
                    xt = ld_pool.tile([128, E], F32, tag="xld")
                    nc.sync.dma_start(xt[:], x_d[sb * 128:(sb + 1) * 128, :])
                    xb = cst_pool.tile([128, E], BF16, tag="xcst")
                    nc.vector.tensor_copy(xb[:], xt[:])
                    for ec in range(4):
                        pst = ps_set.tile([128, 128], F32, tag="pset")
                        nc.tensor.matmul(
                            pst[:], xb[:, ec * 128:(ec + 1) * 128],
                            ident_bf16[:], start=True, stop=True)
                        eng = nc.scalar.copy if ec % 2 else \
                            nc.vector.tensor_copy
                        eng(xT[ec][:, sb * 128:(sb + 1) * 128], pst[:])
                # weights
                for w_name, w_d in (("wq", wq_d), ("wk", wk_d), ("wv", wv_d)):
                    for pc in range(CH // 128):
                        wt = ld_pool.tile([128, E], F32, tag="wld")
                        nc.sync.dma_start(
                            wt[:], w_d[pc * 128:(pc + 1) * 128, :])
                        wb = cst_pool.tile([128, E], BF16, tag="wcst")
                        nc.vector.tensor_copy(wb[:], wt[:])
                        for ec in range(4):
                            pst = ps_set.tile([128, 128], F32, tag="pset")
                            nc.tensor.matmul(
                                pst[:], wb[:, ec * 128:(ec + 1) * 128],
                                ident_bf16[:], start=True, stop=True)
                            eng = nc.scalar.copy if ec % 2 else \
                                nc.vector.tensor_copy
                            eng(wT[w_name][ec][:, pc * 128:(pc + 1) * 128],
                                pst[:])
                # Er: one load+cast, then transpose into both partition halves
                et = ld_pool.tile([128, SB * HD], F32, tag="eld")
                nc.sync.dma_start(
                    et[:].rearrange("p (b d) -> p b d", d=HD),
                    er_d[:].rearrange("(b p) d -> p b d", p=128))
                eb = cst_pool.tile([128, SB * HD], BF16, tag="ecst")
                nc.vector.tensor_copy(eb[:], et[:])
                for sb in range(SB):
                    pst = ps_set.tile([128, 128], F32, tag="pset")
                    nc.tensor.matmul(
                        pst[0:64, :], eb[:, sb * HD:(sb + 1) * HD],
                        ident_bf16[:], start=True, stop=True)
                    nc.tensor.matmul(
                        pst[64:128, :], eb[:, sb * HD:(sb + 1) * HD],
                        ident_bf16[:], start=True, stop=True)
                    nc.vector.tensor_copy(
                        erT[:, sb * 128:(sb + 1) * 128], pst[:])

            # ================= attention =================
            if True:
                def phase_a_pair(hp, sb, first=False):
                    """QEr for heads (2hp, 2hp+1), q-block sb -> P[h] (fp8).

                    The two heads' matmuls go to different row groups
                    (base partitions 0/64) and stream concurrently."""
                    pex = [wrk.tile([128, s + 1], FP8, tag=f"pexp{i}",
                                    name=f"pexp{i}") for i in range(2)]
                    # P's zero column travels with the tile (at the END of
                    # the row -- the whole flat layout is shifted by one
                    # element, keeping the psum drains byte-aligned and the
                    # DRAM write fully contiguous).  The drains never touch
                    # col s, so zeroing each ring buffer once (during the
                    # prologue, which cycles every buf) is enough.
                    if first:
                        nc.vector.memset(pex[0][:, s:s + 1], 0.0)
                        nc.vector.memset(pex[1][:, s:s + 1], 0.0)
                    for lt in range(NLT):
                        psA = ps_m.tile([128, LT], F32, tag="pm", name="psA")
                        psB = ps_m.tile([128, LT], F32, tag="pm", name="psB")
                        for c in range(LT // 512):
                            l0 = lt * LT + c * 512
                            nc.tensor.matmul(
                                psA[:, c * 512:(c + 1) * 512],
                                qT[hp][0:64, sb * 128:(sb + 1) * 128],
                                erT[0:64, l0:l0 + 512],
                                start=True, stop=True)
                            nc.tensor.matmul(
                                psB[:, c * 512:(c + 1) * 512],
                                qT[hp][64:128, sb * 128:(sb + 1) * 128],
                                erT[64:128, l0:l0 + 512],
                                start=True, stop=True)
                        # drains: DVE for head A; head B alternates DVE/ACT
                        # (GPSIMD cannot read PSUM on TRN2)
                        nc.vector.tensor_copy(
                            pex[0][:, lt * LT:(lt + 1) * LT], psA[:])
                        engb = nc.vector.tensor_copy if sb % 2 == 0 else \
                            nc.scalar.copy
                        engb(pex[1][:, lt * LT:(lt + 1) * LT], psB[:])
                    for i in range(2):
                        h = 2 * hp + i
                        base1 = sb * 128 * (s + 1)
                        nc.sync.dma_start(
                            p_ds[h][base1:base1 + 128 * (s + 1)]
                            .rearrange("(r c) -> r c", c=s + 1),
                            pex[i][:])

                def phase_a_row(hp, row):
                    """QEr for the single q-row `row`, both heads of pair
                    hp -> P[h] row `row`.

                    The skewed read of a group's last q-block needs exactly
                    one row of the NEXT group's first block; computing it
                    separately (M=1 matmuls, 2KB bounce) keeps that read off
                    the full block's drain+write chain.  The full block
                    later rewrites the row with identical data."""
                    pexr = [small.tile([1, s + 1], FP8, tag=f"pexr{i}",
                                       name=f"pexr{i}", bufs=2)
                            for i in range(2)]
                    nc.vector.memset(pexr[0][:, s:s + 1], 0.0)
                    nc.vector.memset(pexr[1][:, s:s + 1], 0.0)
                    for lt in range(NLT):
                        psA = ps_m.tile([128, LT], F32, tag="pm", name="psA")
                        psB = ps_m.tile([128, LT], F32, tag="pm", name="psB")
                        for c in range(LT // 512):
                            l0 = lt * LT + c * 512
                            nc.tensor.matmul(
                                psA[0:1, c * 512:(c + 1) * 512],
                                qT[hp][0:64, row:row + 1],
                                erT[0:64, l0:l0 + 512],
                                start=True, stop=True)
                            nc.tensor.matmul(
                                psB[0:1, c * 512:(c + 1) * 512],
                                qT[hp][64:128, row:row + 1],
                                erT[64:128, l0:l0 + 512],
                                start=True, stop=True)
                        nc.vector.tensor_copy(
                            pexr[0][:, lt * LT:(lt + 1) * LT], psA[0:1, :])
                        nc.vector.tensor_copy(
                            pexr[1][:, lt * LT:(lt + 1) * LT], psB[0:1, :])
                    for i in range(2):
                        h = 2 * hp + i
                        base1 = row * (s + 1)
                        nc.sync.dma_start(
                            p_ds[h][base1:base1 + (s + 1)]
                            .rearrange("(r c) -> r c", c=s + 1),
                            pexr[i][:])

                def skew_read(h, sb):
                    """Skewed strided read of P[h] for q-block sb (the -1
                    accounts for the zero column sitting at the end of each
                    P row instead of the front)."""
                    st = wrk2.tile([128, s], FP8, tag="srel", name="srel",
                                   bufs=22)
                    base = (sb * 128 + 1) * s - 1
                    nc.sync.dma_start(
                        st[:],
                        p_ds[h][base:base + 128 * s]
                        .rearrange("(r c) -> r c", c=s))
                    return st

                def tp_step(hp, ssb, tp, srels, ttA, ttB):
                    """Transposed scores for one pair of t-blocks, both
                    heads of pair hp.

                    scoresT[t-block, i] = k_tb^T q (kT-block stationary, q
                    moving at N=512); the heads alternate row groups so the
                    two QK matmuls overlap in the array.  The four srel
                    skew-blocks are transpose-accumulated into the same PSUM
                    group via fp8 identity matmuls; exp drains PSUM straight
                    into the AV-ready ttile slices, [128,1024] per ACT
                    instruction."""
                    q0 = ssb * QG * 128
                    psA = ps_m.tile([128, 1024], F32, tag="pm", name="psA")
                    psB = ps_m.tile([128, 1024], F32, tag="pm", name="psB")
                    for j in range(2):
                        tb = 2 * tp + j
                        nc.tensor.matmul(
                            psA[:, j * 512:(j + 1) * 512],
                            kT[hp][0:64, tb * 128:(tb + 1) * 128],
                            qT[hp][0:64, q0:q0 + 512],
                            start=True, stop=False, skip_group_check=True)
                        nc.tensor.matmul(
                            psB[:, j * 512:(j + 1) * 512],
                            kT[hp][64:128, tb * 128:(tb + 1) * 128],
                            qT[hp][64:128, q0:q0 + 512],
                            start=True, stop=False, skip_group_check=True)
                    for j in range(2):
                        tb = 2 * tp + j
                        for ps, sr in ((psA, srels[0]), (psB, srels[1])):
                            for ib in range(QG):
                                nc.tensor.matmul(
                                    ps[:, j * 512 + ib * 128:
                                       j * 512 + (ib + 1) * 128],
                                    sr[ib][:, tb * 128:(tb + 1) * 128],
                                    ident_fp8[:],
                                    start=False, stop=(ib == QG - 1),
                                    skip_group_check=True)
                    nc.scalar.activation(
                        ttA[:, 2 * tp:2 * tp + 2, :]
                        .rearrange("p a c -> p (a c)"),
                        psA[:],
                        mybir.ActivationFunctionType.Exp, scale=SCALE)
                    nc.scalar.activation(
                        ttB[:, 2 * tp:2 * tp + 2, :]
                        .rearrange("p a c -> p (a c)"),
                        psB[:],
                        mybir.ActivationFunctionType.Exp, scale=SCALE)

                def phase_b_av(hloc, ssb, ttile):
                    """AV burst for one 512-query group; stash raw result."""
                    av_ps = ps_av.tile([HD + 1, 512], F32, tag="av",
                                       name="av_ps")
                    for ci in range(SB):
                        nc.tensor.matmul(
                            av_ps[:],
                            vaug[ci][:, hloc * (HD + 1):
                                     (hloc + 1) * (HD + 1)],
                            ttile[:, ci, :],
                            start=(ci == 0), stop=(ci == SB - 1))
                    avs = avp.tile([HD + 1, 512], BF16, tag="avs",
                                   name="avs")
                    nc.scalar.copy(avs[:], av_ps[:])
                    for q4 in range(QG):
                        sb = ssb * QG + q4
                        avT = ps_avT.tile([128, HD + 1], F32, tag="avT",
                                          name="avT")
                        nc.tensor.matmul(
                            avT[:], avs[:, q4 * 128:(q4 + 1) * 128],
                            ident_bf16[0:HD + 1, 0:HD + 1],
                            start=True, stop=True)
                        nc.vector.tensor_copy(
                            outp[sb][:, hloc * (HD + 1):
                                     (hloc + 1) * (HD + 1)],
                            avT[:])

                def div_block(sb):
                    """One batched reciprocal of the 4 heads' softmax sums
                    (at the 65-col slot tails), then scale the raw AV
                    numerators in place."""
                    o3 = outp[sb][:].rearrange("p (h c) -> p h c", c=HD + 1)
                    rinv4 = small.tile([128, HLOC], F32, tag="rinv4",
                                       name="rinv4")
                    nc.vector.reciprocal(rinv4[:], o3[:, :, HD])
                    for hh in range(HLOC):
                        nc.vector.tensor_scalar_mul(
                            o3[:, hh, 0:HD],
                            o3[:, hh, 0:HD],
                            rinv4[:, hh:hh + 1])

                def ln_stats_block(sb):
                    o3 = outp[sb][:].rearrange("p (h c) -> p h c", c=HD + 1)
                    s1 = small.tile([128, 1], F32, tag="s1", name="s1")
                    nc.vector.reduce_sum(
                        s1[:], o3[:, :, 0:HD], axis=mybir.AxisListType.XY)
                    sq = small.tile([128, 1], F32, tag="sq", name="sq")
                    scr = wrk.tile([128, CH], F32, tag="scr", name="scr")
                    nc.scalar.activation(
                        scr[:], o3[:, :, 0:HD],
                        mybir.ActivationFunctionType.Square, accum_out=sq[:])
                    nc.sync.dma_start(
                        cc_in[sb * 128:(sb + 1) * 128, 0:1], s1[:])
                    nc.sync.dma_start(
                        cc_in[sb * 128:(sb + 1) * 128, 1:2], sq[:])

                def ln_chunk_collective(ssb):
                    r0 = ssb * QG * 128
                    r1 = (ssb + 1) * QG * 128
                    nc.gpsimd.collective_compute(
                        "AllReduce", mybir.AluOpType.add,
                        replica_groups=pairs,
                        ins=[cc_in[r0:r1, :].opt()],
                        outs=[cc_out[r0:r1, :].opt()])

                def ln_apply_block(sb):
                    o3 = outp[sb][:].rearrange("p (h c) -> p h c", c=HD + 1)
                    st = small.tile([128, 2], F32, tag="st")
                    # cc_out read + final write go via the (idle) GPSIMD
                    # DMA queue: on the SP queue they'd block later skew
                    # reads behind the collective's latency
                    nc.gpsimd.dma_start(
                        st[:], cc_out[sb * 128:(sb + 1) * 128, :])
                    me2 = small.tile([128, 2], F32, tag="me2")
                    nc.vector.tensor_scalar_mul(me2[:], st[:], 1.0 / E)
                    msq = small.tile([128, 1], F32, tag="msq")
                    nc.vector.tensor_mul(msq[:], me2[:, 0:1], me2[:, 0:1])
                    var = small.tile([128, 1], F32, tag="var")
                    nc.vector.tensor_scalar(
                        var[:], me2[:, 1:2], msq[:], EPS,
                        op0=mybir.AluOpType.subtract,
                        op1=mybir.AluOpType.add)
                    vrec = small.tile([128, 1], F32, tag="vrec")
                    nc.vector.reciprocal(vrec[:], var[:])
                    rstd = small.tile([128, 1], F32, tag="rstd")
                    nc.scalar.activation(
                        rstd[:], vrec[:],
                        mybir.ActivationFunctionType.Sqrt)
                    tmp = wrk.tile([128, CH], F32, tag="tmp")
                    nc.vector.tensor_scalar(
                        tmp[:].rearrange("p (h c) -> p h c", c=HD),
                        o3[:, :, 0:HD], me2[:, 0:1], rstd[:],
                        op0=mybir.AluOpType.subtract,
                        op1=mybir.AluOpType.mult)
                    y1 = wrk2.tile([128, CH], F32, tag="y1")
                    nc.gpsimd.tensor_mul(y1[:], tmp[:], gamma_bc[:])
                    y2 = wrk.tile([128, CH], F32, tag="y2")
                    nc.gpsimd.tensor_add(y2[:], y1[:], beta_bc[:])
                    nc.gpsimd.dma_start(
                        out_d[sb * 128:(sb + 1) * 128, :], y2[:])

                # ====== projections, interleaved with the QEr prologue =====
                # q first (feeds phase A), then the first group's QEr pairs
                # woven between the k/v projection matmuls so the PE stream
                # stays dense while the QEr drains + DMA round trip complete.
                ps_pj = tc.tile_pool(name="ps_pj", bufs=2, space="PSUM")
                pjp = ps_pj.__enter__()

                def proj_qk(dst, w_name, oc, scs):
                    for sc in scs:
                        ps = pjp.tile([128, 512], F32, tag="pj", name="pj")
                        for ec in range(4):
                            nc.tensor.matmul(
                                ps[:],
                                wT[w_name][ec][:, oc * 128:(oc + 1) * 128],
                                xT[ec][:, sc * 512:(sc + 1) * 512],
                                start=(ec == 0), stop=(ec == 3))
                        eng = nc.scalar.copy if sc % 2 else \
                            nc.vector.tensor_copy
                        eng(dst[oc][:, sc * 512:(sc + 1) * 512], ps[:])

                def proj_v(sbs):
                    for sb in sbs:
                        ps = pjp.tile([128, CH], F32, tag="pj", name="pj")
                        for ec in range(4):
                            nc.tensor.matmul(
                                ps[:],
                                xT[ec][:, sb * 128:(sb + 1) * 128],
                                wT["wv"][ec][:],
                                start=(ec == 0), stop=(ec == 3))
                        for h in range(HLOC):
                            nc.scalar.copy(
                                vaug[sb][:, h * (HD + 1):h * (HD + 1) + HD],
                                ps[:, h * HD:(h + 1) * HD])
                            nc.vector.memset(
                                vaug[sb][:, h * (HD + 1) + HD:
                                         (h + 1) * (HD + 1)],
                                1.0)

                for oc in range(2):
                    proj_qk(qT, "wq", oc, range(KC))
                pend0 = list(range(QG))
                for oc in range(2):
                    for scp in range(0, KC, 2):
                        if pend0:
                            phase_a_pair(0, pend0.pop(0), first=True)
                        proj_qk(kT, "wk", oc, range(scp, min(scp + 2, KC)))
                while pend0:
                    phase_a_pair(0, pend0.pop(0), first=True)
                for vb in range(QG):
                    phase_a_pair(1, vb, first=True)
                    proj_v(range(vb * SB // QG, (vb + 1) * SB // QG))
                ps_pj.__exit__(None, None, None)
                xw_pool.__exit__(None, None, None)
                # LN constants, needed only at the tail of each chunk
                nc.sync.dma_start(gamma_bc[:],
                                  gamma_d[:].broadcast_to((128, CH)))
                nc.sync.dma_start(beta_bc[:],
                                  beta_d[:].broadcast_to((128, CH)))
                wrk2 = att_stk.enter_context(
                    tc.tile_pool(name="wrk2", bufs=4))
                ttp = att_stk.enter_context(tc.tile_pool(name="ttp", bufs=4))
                avp = att_stk.enter_context(tc.tile_pool(name="avp", bufs=2))
                small = att_stk.enter_context(
                    tc.tile_pool(name="small", bufs=8))
                ps_av = att_stk.enter_context(
                    tc.tile_pool(name="ps_av", bufs=1, space="PSUM"))
                ps_avT = att_stk.enter_context(
                    tc.tile_pool(name="ps_avT", bufs=1, space="PSUM"))

                # -------- main attention loop: ssb outer, heads inner ------
                # Software pipeline: the next group's QEr (phase A, PE-light
                # + DVE/ACT drains) is interleaved between this group's
                # score tp-steps (PE-dense, ACT exp drains) so every engine
                # sees work from two streams at all times.  AV bursts for
                # pair 0 run inside pair 1's score loop.
                NTP = SB // 2
                # early skew reads for heads 0/1 of the first group
                nxt_early = {h: [skew_read(h, sb4) for sb4 in range(QG - 1)]
                             for h in range(2)}
                for ssb in range(NSSB):
                    nxt = ssb + 1 < NSSB
                    nb = (ssb + 1) * QG
                    srels = nxt_early
                    # heads 2/3's early reads: their tp-steps run late
                    # enough that issuing here still hides the DMA
                    for h in range(2, HLOC):
                        srels[h] = [skew_read(h, ssb * QG + sb4)
                                    for sb4 in range(QG - 1)]
                    # single-row QEr for the next group's first row, then
                    # the last skew reads (which need only that row of the
                    # next group) -- off the full-block drain chain
                    for hp in range(2):
                        if nxt:
                            phase_a_row(hp, nb * 128)
                        for i in range(2):
                            srels[2 * hp + i].append(
                                skew_read(2 * hp + i, ssb * QG + QG - 1))
                    tts = {}
                    for hp in range(2):
                        ttA = ttp.tile([128, SB, 512], BF16, tag="tt",
                                       name="ttA")
                        ttB = ttp.tile([128, SB, 512], BF16, tag="tt",
                                       name="ttB")
                        tts[hp] = (ttA, ttB)
                        pend_a = list(range(nb, nb + QG)) if nxt else []
                        av_done = [False, False]
                        for tp in range(NTP):
                            tp_step(hp, ssb,  tp,
                                    (srels[2 * hp], srels[2 * hp + 1]),
                                    ttA, ttB)
                            if tp % 2 == 1 and pend_a:
                                phase_a_pair(hp, pend_a.pop(0))
                            if hp == 1 and tp == 2:
                                phase_b_av(0, ssb, tts[0][0])
                                av_done[0] = True
                            if hp == 1 and tp == 5:
                                phase_b_av(1, ssb, tts[0][1])
                                av_done[1] = True
                        while pend_a:
                            phase_a_pair(hp, pend_a.pop(0))
                        if hp == 0:
                            # previous chunk's LN here: by now its
                            # AllReduce has had half an iteration to land,
                            # so the DVE queue isn't blocked at its head
                            if ssb > 0:
                                for sb4 in range(QG):
                                    ln_apply_block((ssb - 1) * QG + sb4)
                        else:
                            if not av_done[0]:
                                phase_b_av(0, ssb, tts[0][0])
                            if not av_done[1]:
                                phase_b_av(1, ssb, tts[0][1])
                    phase_b_av(2, ssb, tts[1][0])
                    phase_b_av(3, ssb, tts[1][1])
                    for sb4 in range(QG):
                        div_block(ssb * QG + sb4)
                        ln_stats_block(ssb * QG + sb4)
                    ln_chunk_collective(ssb)
                    # prefetch heads 0/1's skew reads for the next group
                    # (their P rows were written during this iteration)
                    nxt_early = {}
                    if nxt:
                        nxt_early = {
                            h: [skew_read(h, nb + sb4)
                                for sb4 in range(QG - 1)]
                            for h in range(2)}
                for sb4 in range(QG):
                    ln_apply_block((NSSB - 1) * QG + sb4)
                att_stk.close()

    if legalize:
        _legalize_waits(nc)
    return nc


def _legalize_waits(nc):
    """walrus's codegen accepts at most one sync wait on most instruction
    structs; hoist extra waits onto NoOps inserted just before, on the
    same engine queue (program order preserves the semantics)."""
    n = 0
    keep = set()
    for bb in nc.main_func.blocks:
        out = []
        for inst in bb.instructions:
            si = inst.sync_info
            if (inst.opcode not in keep and si is not None
                    and si.on_wait and len(si.on_wait) > 1):
                for w in si.on_wait[:-1]:
                    nop = mybir.InstNoOp(
                        name=f"I-mmw{n}", ins=[], outs=[])
                    n += 1
                    nop.engine = inst.engine
                    nop.sync_info = mybir.SyncInfo(
                        on_wait=[w], on_update=[])
                    out.append(nop)
                si.on_wait = [si.on_wait[-1]]
            out.append(inst)
        bb.instructions = out


_NC_CACHE = {}


def _get_nc(s=S, n_cores=N_CORES):
    key = (s, n_cores)
    if key not in _NC_CACHE:
        _NC_CACHE[key] = build_nc(s, n_cores)
    return _NC_CACHE[key]


def make_in_maps(x, Wq, Wk, Wv, Er, gamma, beta, n_cores=N_CORES):
    in_maps = []
    for c in range(n_cores):
        b, hg = c // 2, c % 2
        sl = slice(hg * CH, (hg + 1) * CH)
        in_maps.append({
            "x": np.ascontiguousarray(x[b], dtype=np.float32),
            "wq": np.ascontiguousarray(Wq[sl], dtype=np.float32),
            "wk": np.ascontiguousarray(Wk[sl], dtype=np.float32),
            "wv": np.ascontiguousarray(Wv[sl], dtype=np.float32),
            "er": np.ascontiguousarray(Er, dtype=np.float32),
            "gamma": np.ascontiguousarray(gamma[sl], dtype=np.float32)[None, :],
            "beta": np.ascontiguousarray(beta[sl], dtype=np.float32)[None, :],
        })
    return in_maps


def assemble(results, n_cores=N_CORES, s=S):
    y = np.empty((n_cores // 2, s, E), np.float32)
    for c in range(n_cores):
        y[c // 2, :, (c % 2) * CH:(c % 2 + 1) * CH] = results[c]["out"]
    return y


def kernel(**inputs):
    from concourse.bass_utils import run_bass_kernel_spmd
    nc = _get_nc()
    in_maps = make_in_maps(
        inputs["x"], inputs["Wq"], inputs["Wk"], inputs["Wv"],
        inputs["Er"], inputs["gamma"], inputs["beta"])
    res = run_bass_kernel_spmd(nc, in_maps, list(range(N_CORES)))
    return assemble(res.results)


# revision 41
# speedup vs baseline: 1.3223x; 1.0285x over previous
"""Relative-position attention (Music-Transformer style skew) + LayerNorm,
distributed over 8 TRN2 NeuronCores.

Sharding: data-parallel over batch (B=4) x tensor-parallel over head-halves
(H=8 -> 2 groups of 4). Core c handles batch b=c//2, heads [4*(c%2), 4*(c%2)+4),
producing output channels [256*(c%2), +256) of y[b]. The final LayerNorm needs
full-E stats, exchanged via a tiny pairwise AllReduce of (sum, sumsq) per
512-row chunk, overlapped with the next chunk's compute.

Skew trick: Srel[i,j] = F[(i+1)*S + j] where F is the row-major flat view of
the padded matrix P[i, 0]=0, P[i, 1+l]=QEr[i, l] (P is [S, S+1]). We bounce P
through DRAM in fp8-e4m3; the skewed read back is a plain strided DMA.

PE strategy: everything on the PE is bf16/fp8 (enables FWL weight loads and
1 cycle/row at every clock state; fp32 runs multi-pass).  QEr and QK have
contraction dim 64 (head dim), so the two heads of a 128-partition tile are
issued back-to-back to different row groups (base partitions 0/64) and stream
through the array concurrently (~2x).  Srel is injected into the transposed
score PSUM via fp8 identity matmuls (fused transpose + add).  AV uses flipped
operand roles: the V block (65 cols incl. a ones column for the softmax
denominator) is PE-stationary and the exp'd transposed scores stream at N=512.
"""

import numpy as np

import concourse.bass as bass
import concourse.mybir as mybir
from concourse import masks
from concourse.tile import TileContext

F32 = mybir.dt.float32
BF16 = mybir.dt.bfloat16
FP8 = mybir.dt.float8e4

B, S, E, H = 4, 2048, 512, 8
HD = E // H          # 64
HLOC = 4             # heads per core
CH = HLOC * HD       # 256 output channels per core
SCALE = float(E) ** -0.5
EPS = 1e-5
N_CORES = 8
QG = 4               # 128-row q-blocks per 512-query group


def build_nc(s=S, n_cores=N_CORES, debug=False, legalize=True):
    """Build the per-core Bass graph (SPMD: same graph on all cores)."""
    nc = bass.Bass(target_bir_lowering=False, debug=debug)

    SB = s // 128        # number of 128-row blocks
    KC = s // 512        # number of 512-col chunks
    NSSB = SB // QG      # number of 512-query groups
    LT = min(1024, s)    # phase-A psum tile width (2 banks)
    NLT = s // LT

    x_d = nc.declare_dram_parameter("x", [s, E], F32, isOutput=False)
    wq_d = nc.declare_dram_parameter("wq", [CH, E], F32, isOutput=False)
    wk_d = nc.declare_dram_parameter("wk", [CH, E], F32, isOutput=False)
    wv_d = nc.declare_dram_parameter("wv", [CH, E], F32, isOutput=False)
    er_d = nc.declare_dram_parameter("er", [s, HD], F32, isOutput=False)
    gamma_d = nc.declare_dram_parameter("gamma", [1, CH], F32, isOutput=False)
    beta_d = nc.declare_dram_parameter("beta", [1, CH], F32, isOutput=False)
    out_d = nc.declare_dram_parameter("out", [s, CH], F32, isOutput=True)

    # Padded-QEr bounce buffers, one per head, flat [S*(S+1)] fp8.
    p_ds = [nc.dram_tensor(f"pbuf{h}", [s * (s + 1)], FP8)
            for h in range(HLOC)]
    cc_in = nc.dram_tensor("cc_in", [s, 2], F32)
    cc_out = nc.dram_tensor("cc_out", [s, 2], F32)

    pairs = [[2 * i, 2 * i + 1] for i in range(n_cores // 2)]

    with TileContext(nc) as tc:
        with (
            tc.tile_pool(name="const", bufs=1) as const_pool,
            tc.tile_pool(name="persist", bufs=1) as pp,
        ):
            ident_bf16 = const_pool.tile([128, 128], BF16)
            ident_fp8 = const_pool.tile([128, 128], FP8)
            masks.make_identity(nc, ident_bf16[:])
            masks.make_identity(nc, ident_fp8[:])
            # zero operand for HAM warm-up matmuls (see x-load loop)
            warm_src = const_pool.tile([128, 512], BF16)
            nc.vector.memset(warm_src[:], 0.0)
            gamma_bc = const_pool.tile([128, CH], F32)
            beta_bc = const_pool.tile([128, CH], F32)

            # ---- persistent SBUF tensors (all bf16 on the PE paths) ----
            # ErT replicated into both partition halves so each head of a
            # pair finds it at its own base partition.
            erT = pp.tile([128, s], BF16, tag="erT")
            qT = [pp.tile([128, s], BF16, tag=f"qT{oc}", name=f"qT{oc}")
                  for oc in range(2)]
            kT = [pp.tile([128, s], BF16, tag=f"kT{oc}", name=f"kT{oc}")
                  for oc in range(2)]
            # v with a ones column appended per head: [128, HLOC*(HD+1)] bf16
            vaug = [pp.tile([128, HLOC * (HD + 1)], BF16, tag=f"va{sb}",
                            name=f"va{sb}") for sb in range(SB)]
            # raw AV numerators + softmax sum per head (65-col head slots,
            # matching the avT transpose output so it lands in one copy)
            outp = [pp.tile([128, HLOC * (HD + 1)], F32, tag=f"op{sb}",
                            name=f"op{sb}") for sb in range(SB)]

            # Long-lived attention pools open first so short-lived setup
            # pools (xw, ld, cst, ps_set, ps_pj) can close in LIFO order.
            from contextlib import ExitStack
            att_stk = ExitStack()
            wrk = att_stk.enter_context(tc.tile_pool(name="wrk", bufs=4))
            ps_m = att_stk.enter_context(
                tc.tile_pool(name="ps_m", bufs=3, space="PSUM"))

            # ====== setup + projections (xT/wT freed afterwards) ======
            xw_pool = tc.tile_pool(name="xw", bufs=1)
            xwp = xw_pool.__enter__()
            xT = [xwp.tile([128, s], BF16, tag=f"xT{ec}", name=f"xT{ec}")
                  for ec in range(4)]
            wT = {
                w: [xwp.tile([128, CH], BF16, tag=f"{w}T{ec}",
                             name=f"{w}T{ec}") for ec in range(4)]
                for w in ("wq", "wk", "wv")
            }
            with (
                tc.tile_pool(name="ld", bufs=6) as ld_pool,
                tc.tile_pool(name="cst", bufs=6) as cst_pool,
                tc.tile_pool(name="ps_set", bufs=2, space="PSUM") as ps_set,
            ):
                # Warm-up: absorb the Pool (identity-creation) dependency
                # into PE's observed clock.
                warm = ps_set.tile([128, 128], F32, tag="pset")
                nc.tensor.matmul(
                    warm[:], ident_bf16[:], ident_bf16[:],
                    start=True, stop=True)

                # xT[ec][:, i*128:(i+1)*128] = bf16(x[i-block, ec-block]).T
                # The x phase is DMA/cast paced, which leaves PE duty too
                # low for the HAM to unthrottle -- everything after would
                # then run its first ~60us at half clock.  Dummy N=512
                # matmuls on zeros fill the idle slots and warm the clock.
                for sb in range(SB):
                    xt = ld_pool.tile([128, E], F32, tag="xld")
                    nc.sync.dma_start(xt[:], x_d[sb * 128:(sb + 1) * 128, :])
                    xb = cst_pool.tile([128, E], BF16, tag="xcst")
                    nc.vector.tensor_copy(xb[:], xt[:])
                    for w in range(2):
                        pswarm = ps_m.tile([128, LT], F32, tag="pm",
                                           name="pswarm")
                        nc.tensor.matmul(
                            pswarm[:, 0:512], ident_bf16[:], warm_src[:],
                            start=True, stop=True)
                    for ec in range(4):
                        pst = ps_set.tile([128, 128], F32, tag="pset")
                        nc.tensor.matmul(
                            pst[:], xb[:, ec * 128:(ec + 1) * 128],
                            ident_bf16[:], start=True, stop=True)
                        eng = nc.scalar.copy if ec % 2 else \
                            nc.vector.tensor_copy
                        eng(xT[ec][:, sb * 128:(sb + 1) * 128], pst[:])
                # weights
                for w_name, w_d in (("wq", wq_d), ("wk", wk_d), ("wv", wv_d)):
                    for pc in range(CH // 128):
                        wt = ld_pool.tile([128, E], F32, tag="wld")
                        nc.sync.dma_start(
                            wt[:], w_d[pc * 128:(pc + 1) * 128, :])
                        wb = cst_pool.tile([128, E], BF16, tag="wcst")
                        nc.vector.tensor_copy(wb[:], wt[:])
                        for ec in range(4):
                            pst = ps_set.tile([128, 128], F32, tag="pset")
                            nc.tensor.matmul(
                                pst[:], wb[:, ec * 128:(ec + 1) * 128],
                                ident_bf16[:], start=True, stop=True)
                            eng = nc.scalar.copy if ec % 2 else \
                                nc.vector.tensor_copy
                            eng(wT[w_name][ec][:, pc * 128:(pc + 1) * 128],
                                pst[:])
                # Er: one load+cast, then transpose into both partition halves
                et = ld_pool.tile([128, SB * HD], F32, tag="eld")
                nc.sync.dma_start(
                    et[:].rearrange("p (b d) -> p b d", d=HD),
                    er_d[:].rearrange("(b p) d -> p b d", p=128))
                eb = cst_pool.tile([128, SB * HD], BF16, tag="ecst")
                nc.vector.tensor_copy(eb[:], et[:])
                for sb in range(SB):
                    pst = ps_set.tile([128, 128], F32, tag="pset")
                    nc.tensor.matmul(
                        pst[0:64, :], eb[:, sb * HD:(sb + 1) * HD],
                        ident_bf16[:], start=True, stop=True)
                    nc.tensor.matmul(
                        pst[64:128, :], eb[:, sb * HD:(sb + 1) * HD],
                        ident_bf16[:], start=True, stop=True)
                    nc.vector.tensor_copy(
                        erT[:, sb * 128:(sb + 1) * 128], pst[:])

            # ================= attention =================
            if True:
                def phase_a_pair(hp, sb, first=False):
                    """QEr for heads (2hp, 2hp+1), q-block sb -> P[h] (fp8).

                    The two heads' matmuls go to different row groups
                    (base partitions 0/64) and stream concurrently."""
                    pex = [wrk.tile([128, s + 1], FP8, tag=f"pexp{i}",
                                    name=f"pexp{i}") for i in range(2)]
                    # P's zero column travels with the tile (at the END of
                    # the row -- the whole flat layout is shifted by one
                    # element, keeping the psum drains byte-aligned and the
                    # DRAM write fully contiguous).  The drains never touch
                    # col s, so zeroing each ring buffer once (during the
                    # prologue, which cycles every buf) is enough.
                    if first:
                        nc.vector.memset(pex[0][:, s:s + 1], 0.0)
                        nc.vector.memset(pex[1][:, s:s + 1], 0.0)
                    for lt in range(NLT):
                        psA = ps_m.tile([128, LT], F32, tag="pm", name="psA")
                        psB = ps_m.tile([128, LT], F32, tag="pm", name="psB")
                        for c in range(LT // 512):
                            l0 = lt * LT + c * 512
                            nc.tensor.matmul(
                                psA[:, c * 512:(c + 1) * 512],
                                qT[hp][0:64, sb * 128:(sb + 1) * 128],
                                erT[0:64, l0:l0 + 512],
                                start=True, stop=True)
                            nc.tensor.matmul(
                                psB[:, c * 512:(c + 1) * 512],
                                qT[hp][64:128, sb * 128:(sb + 1) * 128],
                                erT[64:128, l0:l0 + 512],
                                start=True, stop=True)
                        # drains: DVE for head A; head B alternates DVE/ACT
                        # (GPSIMD cannot read PSUM on TRN2)
                        nc.vector.tensor_copy(
                            pex[0][:, lt * LT:(lt + 1) * LT], psA[:])
                        engb = nc.vector.tensor_copy if sb % 2 == 0 else \
                            nc.scalar.copy
                        engb(pex[1][:, lt * LT:(lt + 1) * LT], psB[:])
                    for i in range(2):
                        h = 2 * hp + i
                        base1 = sb * 128 * (s + 1)
                        nc.sync.dma_start(
                            p_ds[h][base1:base1 + 128 * (s + 1)]
                            .rearrange("(r c) -> r c", c=s + 1),
                            pex[i][:])

                def phase_a_row(hp, row):
                    """QEr for the single q-row `row`, both heads of pair
                    hp -> P[h] row `row`.

                    The skewed read of a group's last q-block needs exactly
                    one row of the NEXT group's first block; computing it
                    separately (M=1 matmuls, 2KB bounce) keeps that read off
                    the full block's drain+write chain.  The full block
                    later rewrites the row with identical data."""
                    pexr = [small.tile([1, s + 1], FP8, tag=f"pexr{i}",
                                       name=f"pexr{i}", bufs=2)
                            for i in range(2)]
                    nc.vector.memset(pexr[0][:, s:s + 1], 0.0)
                    nc.vector.memset(pexr[1][:, s:s + 1], 0.0)
                    for lt in range(NLT):
                        psA = ps_m.tile([128, LT], F32, tag="pm", name="psA")
                        psB = ps_m.tile([128, LT], F32, tag="pm", name="psB")
                        for c in range(LT // 512):
                            l0 = lt * LT + c * 512
                            nc.tensor.matmul(
                                psA[0:1, c * 512:(c + 1) * 512],
                                qT[hp][0:64, row:row + 1],
                                erT[0:64, l0:l0 + 512],
                                start=True, stop=True)
                            nc.tensor.matmul(
                                psB[0:1, c * 512:(c + 1) * 512],
                                qT[hp][64:128, row:row + 1],
                                erT[64:128, l0:l0 + 512],
                                start=True, stop=True)
                        nc.vector.tensor_copy(
                            pexr[0][:, lt * LT:(lt + 1) * LT], psA[0:1, :])
                        nc.vector.tensor_copy(
                            pexr[1][:, lt * LT:(lt + 1) * LT], psB[0:1, :])
                    for i in range(2):
                        h = 2 * hp + i
                        base1 = row * (s + 1)
                        nc.sync.dma_start(
                            p_ds[h][base1:base1 + (s + 1)]
                            .rearrange("(r c) -> r c", c=s + 1),
                            pexr[i][:])

                def skew_read(h, sb):
                    """Skewed strided read of P[h] for q-block sb (the -1
                    accounts for the zero column sitting at the end of each
                    P row instead of the front)."""
                    st = wrk2.tile([128, s], FP8, tag="srel", name="srel",
                                   bufs=22)
                    base = (sb * 128 + 1) * s - 1
                    nc.sync.dma_start(
                        st[:],
                        p_ds[h][base:base + 128 * s]
                        .rearrange("(r c) -> r c", c=s))
                    return st

                def tp_step(hp, ssb, tp, srels, ttA, ttB):
                    """Transposed scores for one pair of t-blocks, both
                    heads of pair hp.

                    scoresT[t-block, i] = k_tb^T q (kT-block stationary, q
                    moving at N=512); the heads alternate row groups so the
                    two QK matmuls overlap in the array.  The four srel
                    skew-blocks are transpose-accumulated into the same PSUM
                    group via fp8 identity matmuls; exp drains PSUM straight
                    into the AV-ready ttile slices, [128,1024] per ACT
                    instruction."""
                    q0 = ssb * QG * 128
                    psA = ps_m.tile([128, 1024], F32, tag="pm", name="psA")
                    psB = ps_m.tile([128, 1024], F32, tag="pm", name="psB")
                    for j in range(2):
                        tb = 2 * tp + j
                        nc.tensor.matmul(
                            psA[:, j * 512:(j + 1) * 512],
                            kT[hp][0:64, tb * 128:(tb + 1) * 128],
                            qT[hp][0:64, q0:q0 + 512],
                            start=True, stop=False, skip_group_check=True)
                        nc.tensor.matmul(
                            psB[:, j * 512:(j + 1) * 512],
                            kT[hp][64:128, tb * 128:(tb + 1) * 128],
                            qT[hp][64:128, q0:q0 + 512],
                            start=True, stop=False, skip_group_check=True)
                    for j in range(2):
                        tb = 2 * tp + j
                        for ps, sr in ((psA, srels[0]), (psB, srels[1])):
                            for ib in range(QG):
                                nc.tensor.matmul(
                                    ps[:, j * 512 + ib * 128:
                                       j * 512 + (ib + 1) * 128],
                                    sr[ib][:, tb * 128:(tb + 1) * 128],
                                    ident_fp8[:],
                                    start=False, stop=(ib == QG - 1),
                                    skip_group_check=True)
                    nc.scalar.activation(
                        ttA[:, 2 * tp:2 * tp + 2, :]
                        .rearrange("p a c -> p (a c)"),
                        psA[:],
                        mybir.ActivationFunctionType.Exp, scale=SCALE)
                    nc.scalar.activation(
                        ttB[:, 2 * tp:2 * tp + 2, :]
                        .rearrange("p a c -> p (a c)"),
                        psB[:],
                        mybir.ActivationFunctionType.Exp, scale=SCALE)

                def phase_b_av(hloc, ssb, ttile):
                    """AV burst for one 512-query group; stash raw result."""
                    av_ps = ps_av.tile([HD + 1, 512], F32, tag="av",
                                       name="av_ps")
                    for ci in range(SB):
                        nc.tensor.matmul(
                            av_ps[:],
                            vaug[ci][:, hloc * (HD + 1):
                                     (hloc + 1) * (HD + 1)],
                            ttile[:, ci, :],
                            start=(ci == 0), stop=(ci == SB - 1))
                    avs = avp.tile([HD + 1, 512], BF16, tag="avs",
                                   name="avs")
                    nc.scalar.copy(avs[:], av_ps[:])
                    for q4 in range(QG):
                        sb = ssb * QG + q4
                        avT = ps_avT.tile([128, HD + 1], F32, tag="avT",
                                          name="avT")
                        nc.tensor.matmul(
                            avT[:], avs[:, q4 * 128:(q4 + 1) * 128],
                            ident_bf16[0:HD + 1, 0:HD + 1],
                            start=True, stop=True)
                        nc.vector.tensor_copy(
                            outp[sb][:, hloc * (HD + 1):
                                     (hloc + 1) * (HD + 1)],
                            avT[:])

                def div_block(sb):
                    """One batched reciprocal of the 4 heads' softmax sums
                    (at the 65-col slot tails), then scale the raw AV
                    numerators in place."""
                    o3 = outp[sb][:].rearrange("p (h c) -> p h c", c=HD + 1)
                    rinv4 = small.tile([128, HLOC], F32, tag="rinv4",
                                       name="rinv4")
                    nc.vector.reciprocal(rinv4[:], o3[:, :, HD])
                    for hh in range(HLOC):
                        nc.vector.tensor_scalar_mul(
                            o3[:, hh, 0:HD],
                            o3[:, hh, 0:HD],
                            rinv4[:, hh:hh + 1])

                def ln_stats_block(sb):
                    o3 = outp[sb][:].rearrange("p (h c) -> p h c", c=HD + 1)
                    stt = small.tile([128, 2], F32, tag="stt", name="stt")
                    nc.vector.reduce_sum(
                        stt[:, 0:1], o3[:, :, 0:HD],
                        axis=mybir.AxisListType.XY)
                    scr = wrk.tile([128, CH], F32, tag="scr", name="scr")
                    nc.scalar.activation(
                        scr[:], o3[:, :, 0:HD],
                        mybir.ActivationFunctionType.Square,
                        accum_out=stt[:, 1:2])
                    nc.sync.dma_start(
                        cc_in[sb * 128:(sb + 1) * 128, :], stt[:])

                def ln_chunk_collective(ssb):
                    r0 = ssb * QG * 128
                    r1 = (ssb + 1) * QG * 128
                    nc.gpsimd.collective_compute(
                        "AllReduce", mybir.AluOpType.add,
                        replica_groups=pairs,
                        ins=[cc_in[r0:r1, :].opt()],
                        outs=[cc_out[r0:r1, :].opt()])

                def ln_apply_block(sb, last=False):
                    o3 = outp[sb][:].rearrange("p (h c) -> p h c", c=HD + 1)
                    st = small.tile([128, 2], F32, tag="st")
                    # cc_out read + final write go via the (idle) GPSIMD
                    # DMA queue: on the SP queue they'd block later skew
                    # reads behind the collective's latency
                    nc.gpsimd.dma_start(
                        st[:], cc_out[sb * 128:(sb + 1) * 128, :])
                    me2 = small.tile([128, 2], F32, tag="me2")
                    nc.vector.tensor_scalar_mul(me2[:], st[:], 1.0 / E)
                    msq = small.tile([128, 1], F32, tag="msq")
                    nc.vector.tensor_mul(msq[:], me2[:, 0:1], me2[:, 0:1])
                    var = small.tile([128, 1], F32, tag="var")
                    nc.vector.tensor_scalar(
                        var[:], me2[:, 1:2], msq[:], EPS,
                        op0=mybir.AluOpType.subtract,
                        op1=mybir.AluOpType.add)
                    vrec = small.tile([128, 1], F32, tag="vrec")
                    nc.vector.reciprocal(vrec[:], var[:])
                    rstd = small.tile([128, 1], F32, tag="rstd")
                    nc.scalar.activation(
                        rstd[:], vrec[:],
                        mybir.ActivationFunctionType.Sqrt)
                    tmp = wrk.tile([128, CH], F32, tag="tmp")
                    nc.vector.tensor_scalar(
                        tmp[:].rearrange("p (h c) -> p h c", c=HD),
                        o3[:, :, 0:HD], me2[:, 0:1], rstd[:],
                        op0=mybir.AluOpType.subtract,
                        op1=mybir.AluOpType.mult)
                    # off the critical path the y-chain rides the idle
                    # GPSIMD; for the final chunk (nothing left to overlap)
                    # the faster DVE shortens the tail
                    emul = nc.vector.tensor_mul if last else \
                        nc.gpsimd.tensor_mul
                    eadd = nc.vector.tensor_add if last else \
                        nc.gpsimd.tensor_add
                    edma = nc.sync.dma_start if last else \
                        nc.gpsimd.dma_start
                    y1 = wrk2.tile([128, CH], F32, tag="y1")
                    emul(y1[:], tmp[:], gamma_bc[:])
                    y2 = wrk.tile([128, CH], F32, tag="y2")
                    eadd(y2[:], y1[:], beta_bc[:])
                    edma(out_d[sb * 128:(sb + 1) * 128, :], y2[:])

                # ====== projections, interleaved with the QEr prologue =====
                # q first (feeds phase A), then the first group's QEr pairs
                # woven between the k/v projection matmuls so the PE stream
                # stays dense while the QEr drains + DMA round trip complete.
                ps_pj = tc.tile_pool(name="ps_pj", bufs=2, space="PSUM")
                pjp = ps_pj.__enter__()

                def proj_qk(dst, w_name, oc, scs):
                    for sc in scs:
                        ps = pjp.tile([128, 512], F32, tag="pj", name="pj")
                        for ec in range(4):
                            nc.tensor.matmul(
                                ps[:],
                                wT[w_name][ec][:, oc * 128:(oc + 1) * 128],
                                xT[ec][:, sc * 512:(sc + 1) * 512],
                                start=(ec == 0), stop=(ec == 3))
                        eng = nc.scalar.copy if sc % 2 else \
                            nc.vector.tensor_copy
                        eng(dst[oc][:, sc * 512:(sc + 1) * 512], ps[:])

                def proj_v(sbs):
                    for sb in sbs:
                        ps = pjp.tile([128, CH], F32, tag="pj", name="pj")
                        for ec in range(4):
                            nc.tensor.matmul(
                                ps[:],
                                xT[ec][:, sb * 128:(sb + 1) * 128],
                                wT["wv"][ec][:],
                                start=(ec == 0), stop=(ec == 3))
                        for h in range(HLOC):
                            nc.scalar.copy(
                                vaug[sb][:, h * (HD + 1):h * (HD + 1) + HD],
                                ps[:, h * HD:(h + 1) * HD])
                            nc.vector.memset(
                                vaug[sb][:, h * (HD + 1) + HD:
                                         (h + 1) * (HD + 1)],
                                1.0)

                for oc in range(2):
                    proj_qk(qT, "wq", oc, range(KC))
                pend0 = list(range(QG))
                for oc in range(2):
                    for scp in range(0, KC, 2):
                        if pend0:
                            phase_a_pair(0, pend0.pop(0), first=True)
                        proj_qk(kT, "wk", oc, range(scp, min(scp + 2, KC)))
                while pend0:
                    phase_a_pair(0, pend0.pop(0), first=True)
                for vb in range(QG):
                    phase_a_pair(1, vb, first=True)
                    proj_v(range(vb * SB // QG, (vb + 1) * SB // QG))
                ps_pj.__exit__(None, None, None)
                xw_pool.__exit__(None, None, None)
                # LN constants, needed only at the tail of each chunk
                nc.sync.dma_start(gamma_bc[:],
                                  gamma_d[:].broadcast_to((128, CH)))
                nc.sync.dma_start(beta_bc[:],
                                  beta_d[:].broadcast_to((128, CH)))
                wrk2 = att_stk.enter_context(
                    tc.tile_pool(name="wrk2", bufs=4))
                ttp = att_stk.enter_context(tc.tile_pool(name="ttp", bufs=4))
                avp = att_stk.enter_context(tc.tile_pool(name="avp", bufs=2))
                small = att_stk.enter_context(
                    tc.tile_pool(name="small", bufs=8))
                ps_av = att_stk.enter_context(
                    tc.tile_pool(name="ps_av", bufs=1, space="PSUM"))
                ps_avT = att_stk.enter_context(
                    tc.tile_pool(name="ps_avT", bufs=1, space="PSUM"))

                # -------- main attention loop: ssb outer, heads inner ------
                # Software pipeline: the next group's QEr (phase A, PE-light
                # + DVE/ACT drains) is interleaved between this group's
                # score tp-steps (PE-dense, ACT exp drains) so every engine
                # sees work from two streams at all times.  AV bursts for
                # pair 0 run inside pair 1's score loop.
                NTP = SB // 2
                # early skew reads for heads 0/1 of the first group
                nxt_early = {h: [skew_read(h, sb4) for sb4 in range(QG - 1)]
                             for h in range(2)}
                for ssb in range(NSSB):
                    nxt = ssb + 1 < NSSB
                    nb = (ssb + 1) * QG
                    srels = nxt_early
                    # heads 2/3's early reads: their tp-steps run late
                    # enough that issuing here still hides the DMA
                    for h in range(2, HLOC):
                        srels[h] = [skew_read(h, ssb * QG + sb4)
                                    for sb4 in range(QG - 1)]
                    # single-row QEr for the next group's first row, then
                    # the last skew reads (which need only that row of the
                    # next group) -- off the full-block drain chain
                    for hp in range(2):
                        if nxt:
                            phase_a_row(hp, nb * 128)
                        for i in range(2):
                            srels[2 * hp + i].append(
                                skew_read(2 * hp + i, ssb * QG + QG - 1))
                    tts = {}
                    for hp in range(2):
                        ttA = ttp.tile([128, SB, 512], BF16, tag="tt",
                                       name="ttA")
                        ttB = ttp.tile([128, SB, 512], BF16, tag="tt",
                                       name="ttB")
                        tts[hp] = (ttA, ttB)
                        pend_a = list(range(nb, nb + QG)) if nxt else []
                        av_done = [False, False]
                        for tp in range(NTP):
                            tp_step(hp, ssb,  tp,
                                    (srels[2 * hp], srels[2 * hp + 1]),
                                    ttA, ttB)
                            if tp % 2 == 1 and pend_a:
                                phase_a_pair(hp, pend_a.pop(0))
                            if hp == 1 and tp == 2:
                                phase_b_av(0, ssb, tts[0][0])
                                av_done[0] = True
                            if hp == 1 and tp == 5:
                                phase_b_av(1, ssb, tts[0][1])
                                av_done[1] = True
                        while pend_a:
                            phase_a_pair(hp, pend_a.pop(0))
                        if hp == 0:
                            # previous chunk's LN here: by now its
                            # AllReduce has had half an iteration to land,
                            # so the DVE queue isn't blocked at its head
                            if ssb > 0:
                                for sb4 in range(QG):
                                    ln_apply_block((ssb - 1) * QG + sb4)
                        else:
                            if not av_done[0]:
                                phase_b_av(0, ssb, tts[0][0])
                            if not av_done[1]:
                                phase_b_av(1, ssb, tts[0][1])
                    phase_b_av(2, ssb, tts[1][0])
                    phase_b_av(3, ssb, tts[1][1])
                    for sb4 in range(QG):
                        div_block(ssb * QG + sb4)
                        ln_stats_block(ssb * QG + sb4)
                    ln_chunk_collective(ssb)
                    # prefetch heads 0/1's skew reads for the next group
                    # (their P rows were written during this iteration)
                    nxt_early = {}
                    if nxt:
                        nxt_early = {
                            h: [skew_read(h, nb + sb4)
                                for sb4 in range(QG - 1)]
                            for h in range(2)}
                for sb4 in range(QG):
                    ln_apply_block((NSSB - 1) * QG + sb4, last=True)
                att_stk.close()

    if legalize:
        _legalize_waits(nc)
    return nc


def _legalize_waits(nc):
    """walrus's codegen accepts at most one sync wait on most instruction
    structs; hoist extra waits onto NoOps inserted just before, on the
    same engine queue (program order preserves the semantics)."""
    n = 0
    keep = set()
    for bb in nc.main_func.blocks:
        out = []
        for inst in bb.instructions:
            si = inst.sync_info
            if (inst.opcode not in keep and si is not None
                    and si.on_wait and len(si.on_wait) > 1):
                for w in si.on_wait[:-1]:
                    nop = mybir.InstNoOp(
                        name=f"I-mmw{n}", ins=[], outs=[])
                    n += 1
                    nop.engine = inst.engine
                    nop.sync_info = mybir.SyncInfo(
                        on_wait=[w], on_update=[])
                    out.append(nop)
                si.on_wait = [si.on_wait[-1]]
            out.append(inst)
        bb.instructions = out


_NC_CACHE = {}


def _get_nc(s=S, n_cores=N_CORES):
    key = (s, n_cores)
    if key not in _NC_CACHE:
        _NC_CACHE[key] = build_nc(s, n_cores)
    return _NC_CACHE[key]


def make_in_maps(x, Wq, Wk, Wv, Er, gamma, beta, n_cores=N_CORES):
    in_maps = []
    for c in range(n_cores):
        b, hg = c // 2, c % 2
        sl = slice(hg * CH, (hg + 1) * CH)
        in_maps.append({
            "x": np.ascontiguousarray(x[b], dtype=np.float32),
            "wq": np.ascontiguousarray(Wq[sl], dtype=np.float32),
            "wk": np.ascontiguousarray(Wk[sl], dtype=np.float32),
            "wv": np.ascontiguousarray(Wv[sl], dtype=np.float32),
            "er": np.ascontiguousarray(Er, dtype=np.float32),
            "gamma": np.ascontiguousarray(gamma[sl], dtype=np.float32)[None, :],
            "beta": np.ascontiguousarray(beta[sl], dtype=np.float32)[None, :],
        })
    return in_maps


def assemble(results, n_cores=N_CORES, s=S):
    y = np.empty((n_cores // 2, s, E), np.float32)
    for c in range(n_cores):
        y[c // 2, :, (c % 2) * CH:(c % 2 + 1) * CH] = results[c]["out"]
    return y


def kernel(**inputs):
    from concourse.bass_utils import run_bass_kernel_spmd
    nc = _get_nc()
    in_maps = make_in_maps(
        inputs["x"], inputs["Wq"], inputs["Wk"], inputs["Wv"],
        inputs["Er"], inputs["gamma"], inputs["beta"])
    res = run_bass_kernel_spmd(nc, in_maps, list(range(N_CORES)))
    return assemble(res.results)
